# revision 26
# baseline (speedup 1.0000x reference)
"""Clustered Linformer Attention — Trainium2 Bass kernel, 8 NeuronCores.

Strategy: data-parallel over batch (2 batches/core, no collectives).
Math restructuring (verified vs reference to ~7e-7 in f32):
  - mask is all-ones => cluster c holds positions [32c, 32c+32); the per-head
    gather+einsum projections become  k_proj = AE[h]^T @ k_h  with a host-built
    sparse table AE[h] in [S, P] (score scale folded in), same for v with AF.
  - the 3-kernel conv fusion over scores collapses to 5 "tap" matrices M_t in
    [P, P] (t in -2..2):  scores_conv[s] = sum_t  (q[s+t] @ (k_proj^T @ M_t)).
  - v2: the 5 taps are K-STACKED two-per-matmul: qts stores each head's qT
    twice (partitions 0-63 at shift 0, 64-127 at shift +1), and the tap
    operands T0=[bdt(-2);bdt(-1)], T1=[bdt(0);bdt(+1)], T2=bdt(+2) contract
    over 128/128/64 partitions.  The two heads of a pair run in different PE
    column groups (tile_position col 0 / 64), so the 3 matmuls per head
    overlap pairwise -> ~3 matmul-times for what used to take 5.
  - softmax has no max-subtraction (|scores| <~ 1.6, exp is safe in f32);
    Z = sum_c exp is computed by an all-ones block-diag matmul that also
    broadcasts Z to all 128 partitions, so normalization is one DVE op.

Scheduling: x is DMA'd in column slices so QKV starts early; ae/af cluster
tables are partition-major in DRAM (4KB rows) and prefetched per-pair; each
attention unit is split into scores(+exp) and Z/at(+normalize) halves with a
filler matmul unit between them in PE program order (covers ACT exp latency);
dense output copies alternate ACT/DVE and output DMAs alternate queues.
"""
import sys
import numpy as np
import ml_dtypes

sys.path.insert(0, '/opt/trn_rl_repo')

B, S, D = 16, 2048, 512
H, P, C = 8, 64, 32
DEPTH = D // H           # 64
NCORES = 8
BLOC = B // NCORES       # 2 batches per core
NPAIR = H // 2           # 4 head pairs
SCH = 4                  # s-chunks of 512
SCW = S // SCH           # 512
NJ = S // 128            # 16 s-tiles of 128
NDC = D // 128           # 4 contraction chunks
QW = S + 4               # qts width (2 pad front, 2 back)

_CACHE = {}


def _build_nc():
    import concourse.tile as tile
    from concourse import mybir, bacc

    f32 = mybir.dt.float32
    bf16 = mybir.dt.bfloat16

    nc = bacc.Bacc()
    xT = nc.declare_dram_parameter("xT", [BLOC, D, S], bf16, isOutput=False)
    wq = nc.declare_dram_parameter("wq", [D, D], bf16, isOutput=False)
    wk = nc.declare_dram_parameter("wk", [D, D], bf16, isOutput=False)
    wv = nc.declare_dram_parameter("wv", [D, D], bf16, isOutput=False)
    dw = nc.declare_dram_parameter("dw", [D, D], bf16, isOutput=False)
    # compact cluster tables: the dense AE_pair[pr][s=128j+p, c] block j is
    # nonzero only at cols {4j + p//32, 64 + 4j + p//32}; aec[pr, p, j, 0:4]
    # holds cols 4j..4j+3 (head 0), [4:8] cols 64+4j..64+4j+3 (head 1).
    aec = nc.declare_dram_parameter("aec", [NPAIR, 128, NJ, 8], bf16,
                                    isOutput=False)
    afc = nc.declare_dram_parameter("afc", [NPAIR, 128, NJ, 8], bf16,
                                    isOutput=False)
    # per-head tap matrices, duplicated on both partition halves, grouped
    # by K-stack destination: bdma[p, k, c'] = M_{(-2,0,+2)[k]}[p % 64, c']
    # (psum rows 0-63), bdmb[p, k, c'] = M_{(-1,+1)[k]} (psum rows 64-127).
    bdma = nc.declare_dram_parameter("bdma", [128, 3, 64], bf16, isOutput=False)
    bdmb = nc.declare_dram_parameter("bdmb", [128, 2, 64], bf16, isOutput=False)
    onesbd = nc.declare_dram_parameter("onesbd", [128, 128], bf16,
                                       isOutput=False)
    out = nc.declare_dram_parameter("out", [BLOC, S, D], bf16, isOutput=True)

    with tile.TileContext(nc) as tc:
        with tc.tile_pool(name="const", bufs=1) as cpool, \
             tc.tile_pool(name="big", bufs=1) as bigp, \
             tc.tile_pool(name="sm", bufs=4) as smp, \
             tc.tile_pool(name="bd", bufs=10) as bdp, \
             tc.tile_pool(name="ob", bufs=2) as obp, \
             tc.tile_pool(name="psB", bufs=6, space="PSUM") as psB, \
             tc.tile_pool(name="psS", bufs=2, space="PSUM") as psS:

            # ---- constants in SBUF ----
            wq_sb = cpool.tile([128, NDC, D], bf16)
            wk_sb = cpool.tile([128, NDC, D], bf16)
            wv_sb = cpool.tile([128, NDC, D], bf16)
            dw_sb = cpool.tile([128, NDC, D], bf16)
            for t_sb, t_dr in ((wk_sb, wk), (wv_sb, wv), (wq_sb, wq),
                               (dw_sb, dw)):
                nc.sync.dma_start(out=t_sb, in_=t_dr[:].rearrange("(o p) m -> p o m", p=128))
            bdma_sb = cpool.tile([128, 3, 64], bf16)
            nc.sync.dma_start(out=bdma_sb, in_=bdma[:])
            bdmb_sb = cpool.tile([128, 2, 64], bf16)
            nc.sync.dma_start(out=bdmb_sb, in_=bdmb[:])
            ones_sb = cpool.tile([128, 128], bf16)
            nc.sync.dma_start(out=ones_sb, in_=onesbd[:])

            # expanded cluster tables live in 4 persistent SBUF tiles, zeroed
            # once; per-pair fetches only rewrite the 8 value columns per
            # j-block.  Writes go through a stride-136 view (so block j's
            # values land at flat col 136j + {0..3, 64..67}), reads through a
            # stride-132 view whose block j = flat cols [132j, 132j+128) —
            # within it the values sit at in-block cols 4j+{p//32} and
            # 64+4j+{p//32}, exactly the dense table layout.
            AEW = 16 * 136
            aexp = {}
            for nm in ("aeA", "aeB", "afA", "afB"):
                t = cpool.tile([128, AEW], bf16, name=f"aexp_{nm}")
                aexp[nm] = t
                eng = nc.gpsimd if nm[-1] == "A" else nc.vector
                eng.memset(t, 0.0)

            st = [dict() for _ in range(BLOC)]

            def emit_x_load(b):
                # column-sliced so kv/qt of early s-chunks start ASAP;
                # dc slices split across two DMA queues to double feed rate.
                s = st[b]
                s["xt"] = [bigp.tile([128, S], bf16, tag="xt", bufs=2 * NDC,
                                     name=f"xt_{b}_{dc}")
                           for dc in range(NDC)]
                for q in range(SCH):
                    for dc in range(NDC):
                        eng = nc.gpsimd if dc % 2 == 0 else nc.scalar
                        eng.dma_start(
                            out=s["xt"][dc][:, SCW * q:SCW * (q + 1)],
                            in_=xT[b, 128 * dc:128 * (dc + 1),
                                   SCW * q:SCW * (q + 1)])

            def emit_kv(b, j):
                s = st[b]
                if j == 0:
                    s["knat"] = bigp.tile([128, NJ, D], bf16, tag="knat",
                                          name=f"knat_{b}")
                    s["vnat"] = bigp.tile([128, NJ, D], bf16, tag="vnat",
                                          name=f"vnat_{b}")
                for w_sb, key in ((wk_sb, "knat"), (wv_sb, "vnat")):
                    ps_k = psB.tile([128, D], f32, tag="ps512")
                    for dc in range(NDC):
                        nc.tensor.matmul(
                            ps_k,
                            s["xt"][dc][:, 128 * j:128 * (j + 1)],
                            w_sb[:, dc, :],
                            start=(dc == 0), stop=(dc == NDC - 1))
                    if key == "knat":
                        nc.vector.tensor_copy(out=s[key][:, j, :], in_=ps_k)
                    else:
                        nc.scalar.copy(out=s[key][:, j, :], in_=ps_k)

            def emit_qt(b, pr, n):
                # qts[h]: per-head duplicated qT.  partitions 0-63 hold
                # q[d, col-2], partitions 64-127 hold q[d, col-1].
                s = st[b]
                if pr == 0 and n == 0:
                    s["qts"] = bigp.tile([128, H, QW], bf16, tag="qts",
                                         bufs=2, name=f"qts_{b}")
                    nc.vector.memset(s["qts"][0:64, :, 0:2], 0.0)
                    nc.vector.memset(s["qts"][0:64, :, S + 2:], 0.0)
                    nc.vector.memset(s["qts"][64:128, :, 0:1], 0.0)
                    nc.vector.memset(s["qts"][64:128, :, S + 1:], 0.0)
                ps_q = psB.tile([128, SCW], f32, tag="ps512")
                for dc in range(NDC):
                    nc.tensor.matmul(
                        ps_q,
                        wq_sb[:, dc, 128 * pr:128 * (pr + 1)],
                        s["xt"][dc][:, SCW * n:SCW * (n + 1)],
                        start=(dc == 0), stop=(dc == NDC - 1))
                h0 = 2 * pr
                qts = s["qts"]
                b0 = 2 + SCW * n       # block0 col of s = SCW*n
                # one full-width ACT copy: h0 rows into h0 slot block0;
                # h1 rows land in h0 slot's upper half (staging).
                nc.scalar.copy(out=qts[:, h0, b0:b0 + SCW], in_=ps_q)

            def emit_qdup(b, pr):
                # after all 4 chunks of this pair are staged, build the
                # duplicated layouts with 3 full-width SBUF->SBUF DMAs
                # (same queue => in order; sync queue keeps these off the
                # x-feed queues).
                s = st[b]
                qts = s["qts"]
                h0, h1 = 2 * pr, 2 * pr + 1
                #  a) h1 block1 <- staged h1 rows (same partitions, col -1)
                nc.sync.dma_start(out=qts[64:128, h1, 1:1 + S],
                                    in_=qts[64:128, h0, 2:2 + S])
                #  b) h1 block0 <- staged h1 rows (cross partition 64->0)
                nc.sync.dma_start(out=qts[0:64, h1, 2:2 + S],
                                    in_=qts[64:128, h0, 2:2 + S])
                #  c) h0 block1 <- h0 block0 (cross 0->64, col -1); clobbers
                #     the staging except its last column...
                nc.sync.dma_start(out=qts[64:128, h0, 1:1 + S],
                                    in_=qts[0:64, h0, 2:2 + S])
                #     ... and its last column must read q(S)=0: copy the
                #     always-zero block1 col 0 (same queue => after a/b).
                nc.sync.dma_start(out=qts[64:128, h0, S + 1:S + 2],
                                    in_=qts[64:128, h0, 0:1])

            def emit_proj_fetch(b, pr):
                s = st[b]
                ab = "A" if pr % 2 == 0 else "B"
                ae_t, af_t = aexp["ae" + ab], aexp["af" + ab]
                s.setdefault("aef", {})[pr] = (ae_t, af_t)
                for t, src in ((ae_t, aec), (af_t, afc)):
                    w = t.rearrange("p (j c) -> p j c", c=136)
                    nc.sync.dma_start(out=w[:, :, 0:4], in_=src[pr, :, :, 0:4])
                    nc.sync.dma_start(out=w[:, :, 64:68], in_=src[pr, :, :, 4:8])

            def emit_proj(b, pr):
                s = st[b]
                if pr == 0:
                    s["kp"] = bigp.tile([128, NPAIR, 128], bf16, tag="kpbd",
                                        bufs=2, name=f"kp_{b}")
                    s["vp"] = bigp.tile([128, NPAIR, 128], bf16, tag="vpbd",
                                        bufs=2, name=f"vp_{b}")
                    nc.vector.memset(s["vp"], 0.0)
                ae_t, af_t = s["aef"].pop(pr)
                for a_sb, key, dstk in ((ae_t, "knat", "kp"),
                                        (af_t, "vnat", "vp")):
                    a_rd = a_sb[:, 0:16 * 132].rearrange(
                        "p (j c) -> p j c", c=132)
                    ps_p = psS.tile([128, 128], f32, tag="pssmall")
                    for j in range(NJ):
                        nc.tensor.matmul(
                            ps_p,
                            a_rd[:, j, 0:128],
                            st[b][key][:, j, 128 * pr:128 * (pr + 1)],
                            start=(j == 0), stop=(j == NJ - 1))
                    dst = st[b][dstk]
                    if dstk == "kp":
                        # only diag blocks are ever read (per-head lhsT)
                        nc.vector.tensor_copy(out=dst[:, pr, :], in_=ps_p)
                    else:
                        # vp is used as a block-diag [c,d] operand: keep
                        # off-diag zero.
                        nc.vector.tensor_copy(
                            out=dst[0:64, pr, 0:64], in_=ps_p[0:64, 0:64])
                        nc.vector.tensor_copy(
                            out=dst[64:128, pr, 64:128],
                            in_=ps_p[64:128, 64:128])

            def emit_kt(b, pr):
                # per-head K-stacked tap operands:
                # T[:,0]=[bdt(-2);bdt(-1)], T[:,1]=[bdt(0);bdt(+1)],
                # T[0:64,2]=bdt(+2), where bdt(t) = kp_h^T @ M_t  [d, c'].
                s = st[b]
                if pr == 0:
                    s["bdts"] = {}
                    s["expt"] = {}
                    s["cw"] = {}
                for h2 in (0, 1):
                    hb = 64 * h2
                    h = 2 * pr + h2
                    kp_h = s["kp"][hb:hb + 64, pr, hb:hb + 64]
                    T = bdp.tile([128, 3, 64], bf16, tag="bdts",
                                 name=f"bdts_{b}_{h}")
                    s["bdts"][h] = T
                    ps_b = psS.tile([128, 3, 64], f32, tag="pssmall")
                    nc.tensor.matmul(ps_b[0:64, :, :], kp_h,
                                     bdma_sb[hb:hb + 64, :, :],
                                     start=True, stop=True)
                    nc.tensor.matmul(ps_b[64:128, 0:2, :], kp_h,
                                     bdmb_sb[hb:hb + 64, :, :],
                                     start=True, stop=True)
                    nc.scalar.copy(out=T[:, 0:2, :], in_=ps_b[:, 0:2, :])
                    nc.scalar.copy(out=T[0:64, 2, :], in_=ps_b[0:64, 2, :])

            def emit_scores(b, pr, n):
                # 3 K-stacked tap matmuls per head; heads in different PE
                # column groups so consecutive pairs overlap.
                s = st[b]
                qts = s["qts"]
                ps_sc = psB.tile([128, SCW], f32, tag="ps512")
                base = SCW * n
                for h2 in (0, 1):
                    hb = 64 * h2
                    h = 2 * pr + h2
                    T = s["bdts"][h]
                    nc.tensor.matmul(ps_sc[hb:hb + 64, :], T[:, 0, :],
                                     qts[:, h, base:base + SCW],
                                     start=True, stop=False)
                for h2 in (0, 1):
                    hb = 64 * h2
                    h = 2 * pr + h2
                    T = s["bdts"][h]
                    nc.tensor.matmul(ps_sc[hb:hb + 64, :], T[:, 1, :],
                                     qts[:, h, base + 2:base + 2 + SCW],
                                     start=False, stop=False)
                for h2 in (0, 1):
                    hb = 64 * h2
                    h = 2 * pr + h2
                    T = s["bdts"][h]
                    nc.tensor.matmul(ps_sc[hb:hb + 64, :], T[0:64, 2, :],
                                     qts[0:64, h, base + 4:base + 4 + SCW],
                                     start=False, stop=True)
                expt = smp.tile([128, SCW], bf16, tag="expt", bufs=5)
                nc.scalar.activation(
                    out=expt, in_=ps_sc,
                    func=mybir.ActivationFunctionType.Exp)
                s["expt"][(pr, n)] = expt

            def emit_zat(b, pr, n):
                s = st[b]
                expt = s["expt"].pop((pr, n))
                if pr == 0:
                    s["cw"][n] = bigp.tile([128, NPAIR, SCW], bf16,
                                           tag="cwin", bufs=6,
                                           name=f"cw_{b}_{n}")
                ps_z = psB.tile([128, SCW], f32, tag="ps512")
                nc.tensor.matmul(ps_z, ones_sb, expt, start=True, stop=True)
                ps_at = psB.tile([128, SCW], f32, tag="ps512")
                nc.tensor.matmul(ps_at, s["vp"][:, pr, :], expt,
                                 start=True, stop=True)
                rzb = smp.tile([128, SCW], f32, tag="rzb", bufs=2)
                nc.vector.reciprocal_approx_fast(out=rzb, in_=ps_z)
                nc.vector.tensor_mul(
                    out=s["cw"][n][:, pr, :], in0=ps_at, in1=rzb)

            def emit_dense(b, j):
                s = st[b]
                n, jj = j // 4, j % 4
                cw = s["cw"][n]
                ps_d = psB.tile([128, D], f32, tag="ps512")
                for dc in range(NDC):
                    nc.tensor.matmul(
                        ps_d,
                        cw[:, dc, 128 * jj:128 * (jj + 1)],
                        dw_sb[:, dc, :],
                        start=(dc == 0), stop=(dc == NDC - 1))
                emit_dense_out(b, j, ps_d)

            def emit_dense_out(b, j, ps_d):
                obuf = obp.tile([128, D], bf16, tag="obuf")
                if j % 2:
                    nc.vector.tensor_copy(out=obuf, in_=ps_d)
                else:
                    nc.scalar.copy(out=obuf, in_=ps_d)
                eng = (nc.sync, nc.gpsimd, nc.scalar)[j % 3]
                eng.dma_start(out=out[b, 128 * j:128 * (j + 1), :], in_=obuf)

            def emit_dense_partial(b, j, dcs, ps_d):
                # progressive tail dense: accumulate listed dc chunks of
                # output tile j; finish (copy+DMA) when dc 3 lands.
                s = st[b]
                n, jj = j // 4, j % 4
                cw = s["cw"][n]
                for dc in dcs:
                    nc.tensor.matmul(
                        ps_d,
                        cw[:, dc, 128 * jj:128 * (jj + 1)],
                        dw_sb[:, dc, :],
                        start=(dc == 0), stop=(dc == NDC - 1))
                if dcs[-1] == NDC - 1:
                    emit_dense_out(b, j, ps_d)

            # ================= emission schedule =================
            from collections import deque

            # Phase A: batch-0 GEMMs per x-slice quarter; batch-1 x DMAs
            # queued right behind batch-0's.
            emit_x_load(0)
            emit_x_load(1)
            for q in range(SCH):
                for j in range(4 * q, 4 * q + 4):
                    emit_kv(0, j)
                for pr in range(NPAIR):
                    emit_qt(0, pr, q)
                if q == 2:
                    emit_proj_fetch(0, 0)
                    emit_proj_fetch(0, 1)
            for pr in range(NPAIR):
                emit_qdup(0, pr)
            emit_proj(0, 0)
            emit_proj_fetch(0, 2)
            emit_qt(1, 0, 0)
            emit_kt(0, 0)
            emit_proj(0, 1)
            emit_proj_fetch(0, 3)
            emit_qt(1, 0, 1)
            emit_kt(0, 1)
            emit_proj(0, 2)
            emit_qt(1, 0, 2)
            emit_kt(0, 2)
            emit_proj(0, 3)
            emit_qt(1, 0, 3)
            emit_kt(0, 3)
            emit_qdup(1, 0)

            # Phase B: batch-0 attention with batch-1 GEMM units as fillers.
            qt_units = []
            for pr in range(1, NPAIR):
                qt_units += [(emit_qt, (1, pr, n)) for n in range(SCH)]
                qt_units.append((emit_qdup, (1, pr)))
            fill = deque(
                [(emit_kv, (1, j)) for j in range(8)] +
                [(emit_proj_fetch, (1, 0))] +
                [(emit_kv, (1, j)) for j in range(8, NJ)] +
                [(emit_proj_fetch, (1, 1))] +
                qt_units +
                [(emit_proj, (1, 0)), (emit_proj_fetch, (1, 2)),
                 (emit_kt, (1, 0)),
                 (emit_proj, (1, 1)), (emit_proj_fetch, (1, 3)),
                 (emit_kt, (1, 1)),
                 (emit_proj, (1, 2)), (emit_kt, (1, 2)),
                 (emit_proj, (1, 3)), (emit_kt, (1, 3))])

            def popf():
                # emit filler units until one with PE work was emitted
                while fill:
                    f, a = fill.popleft()
                    f(*a)
                    if f not in (emit_proj_fetch, emit_qdup):
                        break

            # scores runs one unit ahead of zat so the ACT exp latency is
            # always covered by the next unit's matmuls even with no fillers.
            prev = None
            for pr in range(NPAIR):
                for n in range(SCH):
                    emit_scores(0, pr, n)
                    popf()
                    if prev is not None:
                        emit_zat(0, *prev)
                        popf()
                    prev = (pr, n)
            emit_zat(0, *prev)
            while fill:
                popf()

            # Phase C: batch-1 attention (scores pipelined one ahead of zat);
            # fillers are batch-0 dense then batch-1 dense as chunks complete.
            # The last chunk's dense is accumulated progressively per pair so
            # only the dc=3 matmuls remain after the final zat.
            fill = deque([(emit_dense, (0, j)) for j in range(NJ)])
            prev = None
            ps_tail = None
            for n in range(SCH):
                for pr in range(NPAIR):
                    emit_scores(1, pr, n)
                    popf()
                    if prev is not None:
                        emit_zat(1, *prev)
                        popf()
                        if prev[1] == SCH - 1 and prev[0] == 2:
                            # pairs 0-2 of the last chunk are done: run their
                            # dense contributions now (dc = pair index).
                            ps_tail = [psB.tile([128, D], f32, tag="ps512",
                                                name=f"ps_tail_{jj}")
                                       for jj in range(4)]
                            for jj, ps_d in enumerate(ps_tail):
                                emit_dense_partial(1, 12 + jj, (0, 1, 2), ps_d)
                    prev = (pr, n)
                if n < SCH - 1:
                    for j in range(4 * n, 4 * n + 4):
                        fill.append((emit_dense, (1, j)))
            emit_zat(1, *prev)
            for jj, ps_d in enumerate(ps_tail):
                emit_dense_partial(1, 12 + jj, (3,), ps_d)
            while fill:
                popf()

    nc.finalize()
    return nc


def _prep_inputs(x, mask, wq, wk, wv, EW, FW, conv_w1, conv_w3, conv_w5, conv_b,
                 dense_w, dense_b, cluster_table):
    """Host-side restructuring -> per-core input maps."""
    bf = ml_dtypes.bfloat16
    x = np.ascontiguousarray(np.asarray(x, np.float32))
    mask = np.asarray(mask)
    counts = np.clip(mask.astype(np.int64).sum(1), 1, S)
    pos = np.asarray(cluster_table)[counts - 1]          # [B, P, C]
    if not (pos == pos[0]).all():
        raise NotImplementedError("per-batch cluster tables not supported")
    p0 = pos[0]                                          # [P, C]

    scale = 1.0 / np.sqrt(np.float32(DEPTH))
    s_idx = p0.ravel()
    c_idx = np.repeat(np.arange(P), C)

    def build_table(W, sc):
        A = np.zeros((H, S + 1, P), np.float32)
        np.add.at(A, (np.arange(H)[:, None], s_idx[None, :], c_idx[None, :]),
                  np.asarray(W, np.float32).reshape(H, P * C) * sc)
        return np.ascontiguousarray(A[:, :S, :])

    AE = build_table(EW, scale)
    AF = build_table(FW, 1.0)
    # pack adjacent heads side by side: [NPAIR, S, 128]
    AE = np.ascontiguousarray(
        AE.reshape(NPAIR, 2, S, P).transpose(0, 2, 1, 3).reshape(NPAIR, S, 128))
    AF = np.ascontiguousarray(
        AF.reshape(NPAIR, 2, S, P).transpose(0, 2, 1, 3).reshape(NPAIR, S, 128))
    # partition-major for fast DMA: [NPAIR, 128, NJ, 128]
    AE = np.ascontiguousarray(
        AE.reshape(NPAIR, NJ, 128, 128).transpose(0, 2, 1, 3))
    AF = np.ascontiguousarray(
        AF.reshape(NPAIR, NJ, 128, 128).transpose(0, 2, 1, 3))

    def compact(A):
        # keep only block j's cols {4j..4j+3, 64+4j..64+4j+3}; valid for the
        # contiguous 32-wide clusters of a full mask.
        out = np.zeros((NPAIR, 128, NJ, 8), np.float32)
        chk = np.zeros_like(A)
        for j in range(NJ):
            out[:, :, j, 0:4] = A[:, :, j, 4 * j:4 * j + 4]
            out[:, :, j, 4:8] = A[:, :, j, 64 + 4 * j:64 + 4 * j + 4]
            chk[:, :, j, 4 * j:4 * j + 4] = out[:, :, j, 0:4]
            chk[:, :, j, 64 + 4 * j:64 + 4 * j + 4] = out[:, :, j, 4:8]
        if not np.array_equal(chk, A):
            raise NotImplementedError("non-contiguous clusters")
        return out

    AEC, AFC = compact(AE), compact(AF)

    # conv -> 5 tap matrices (per-head [P, P], duplicated on both halves)
    wp = np.arange(P)[:, None]
    jj = np.arange(P)[None, :]
    ii = wp - jj + 31
    valid = (ii >= 0) & (ii < P)
    ii = np.clip(ii, 0, P - 1)
    M = {t: np.zeros((P, P), np.float32) for t in range(-2, 3)}
    for cw, hk in ((conv_w1, 1), (conv_w3, 3), (conv_w5, 5)):
        cw = np.asarray(cw, np.float32)
        pad = (hk - 1) // 2
        for dy in range(hk):
            filt = cw[dy, :, 0, 0]
            M[dy - pad] += np.where(valid, filt[ii], 0.0) / 3.0
    BDMA = np.zeros((128, 3, P), np.float32)
    for k, t in enumerate((-2, 0, 2)):
        BDMA[:64, k, :] = M[t]
        BDMA[64:, k, :] = M[t]
    BDMB = np.zeros((128, 2, P), np.float32)
    for k, t in enumerate((-1, 1)):
        BDMB[:64, k, :] = M[t]
        BDMB[64:, k, :] = M[t]
    bbar = float(np.asarray(conv_b, np.float32).mean())
    if abs(bbar) > 1e-30:
        raise NotImplementedError("nonzero conv bias not folded")

    ones_bd = np.zeros((128, 128), np.float32)
    ones_bd[:64, :64] = 1.0
    ones_bd[64:, 64:] = 1.0

    # shard + transpose x
    xsh = x.reshape(NCORES, BLOC, S, D)
    in_maps = []
    shared = dict(
        wq=np.asarray(wq, np.float32).astype(bf),
        wk=np.asarray(wk, np.float32).astype(bf),
        wv=np.asarray(wv, np.float32).astype(bf),
        dw=np.asarray(dense_w, np.float32).astype(bf),
        aec=AEC.astype(bf), afc=AFC.astype(bf),
        bdma=BDMA.astype(bf), bdmb=BDMB.astype(bf),
        onesbd=ones_bd.astype(bf),
    )
    for c in range(NCORES):
        m = dict(shared)
        m["xT"] = np.ascontiguousarray(xsh[c].transpose(0, 2, 1)).astype(bf)
        in_maps.append(m)
    return in_maps


def _run(in_maps, trace=False, tmpdir=None):
    from concourse.bass_utils import run_bass_kernel_spmd
    if "nc" not in _CACHE:
        _CACHE["nc"] = _build_nc()
    kw = {}
    if trace:
        _install_ntff_hook()
        kw = dict(trace=True, tmpdir=tmpdir)
    return run_bass_kernel_spmd(_CACHE["nc"], in_maps,
                                core_ids=list(range(NCORES)), **kw)


def _install_ntff_hook():
    import types, importlib.util as ilu
    if "antenv.axon_hooks" in sys.modules:
        return
    spec = ilu.spec_from_file_location(
        "trn_boot_mod", "/root/.axon_site/trn_agent_boot/trn_boot.py")
    tb = ilu.module_from_spec(spec)
    spec.loader.exec_module(tb)
    hook = tb._ntff_profile_via_ctypes("/opt/axon/libaxon_pjrt.so")
    mod = types.ModuleType("antenv.axon_hooks")
    mod.get_axon_ntff_profile_hook = lambda: hook
    import antenv  # noqa: F401
    sys.modules["antenv.axon_hooks"] = mod


def kernel(**inputs) -> np.ndarray:
    in_maps = _prep_inputs(**inputs)
    r = _run(in_maps)
    out = np.concatenate([np.asarray(r.results[c]["out"], np.float32)
                          for c in range(NCORES)], axis=0)
    db = np.asarray(inputs["dense_b"], np.float32)
    if np.any(db):  # dense bias applied host-side (zero in practice)
        out = out + db
    return out



# revision 33
# speedup vs baseline: 1.0711x; 1.0711x over previous
"""Clustered Linformer Attention — Trainium2 Bass kernel, 8 NeuronCores.

Strategy: data-parallel over batch (2 batches/core, no collectives).
Math restructuring (verified vs reference to ~7e-7 in f32):
  - mask is all-ones => cluster c holds positions [32c, 32c+32); the per-head
    gather+einsum projections become  k_proj = AE[h]^T @ k_h  with a host-built
    sparse table AE[h] in [S, P] (score scale folded in), same for v with AF.
  - the 3-kernel conv fusion over scores collapses to 5 "tap" matrices M_t in
    [P, P] (t in -2..2):  scores_conv[s] = sum_t  (q[s+t] @ (k_proj^T @ M_t)).
  - v2: the 5 taps are K-STACKED two-per-matmul: qts stores each head's qT
    twice (partitions 0-63 at shift 0, 64-127 at shift +1), and the tap
    operands T0=[bdt(-2);bdt(-1)], T1=[bdt(0);bdt(+1)], T2=bdt(+2) contract
    over 128/128/64 partitions.  The two heads of a pair run in different PE
    column groups (tile_position col 0 / 64), so the 3 matmuls per head
    overlap pairwise -> ~3 matmul-times for what used to take 5.
  - softmax has no max-subtraction (|scores| <~ 1.6, exp is safe in f32);
    Z = sum_c exp is computed by an all-ones block-diag matmul that also
    broadcasts Z to all 128 partitions, so normalization is one DVE op.

Scheduling: x is DMA'd in column slices so QKV starts early; ae/af cluster
tables are partition-major in DRAM (4KB rows) and prefetched per-pair; each
attention unit is split into scores(+exp) and Z/at(+normalize) halves with a
filler matmul unit between them in PE program order (covers ACT exp latency);
dense output copies alternate ACT/DVE and output DMAs alternate queues.
"""
import sys
import numpy as np
import ml_dtypes

sys.path.insert(0, '/opt/trn_rl_repo')

B, S, D = 16, 2048, 512
H, P, C = 8, 64, 32
DEPTH = D // H           # 64
NCORES = 8
BLOC = B // NCORES       # 2 batches per core
NPAIR = H // 2           # 4 head pairs
SCH = 4                  # s-chunks of 512
SCW = S // SCH           # 512
NJ = S // 128            # 16 s-tiles of 128
NDC = D // 128           # 4 contraction chunks
QW = S + 4               # qts width (2 pad front, 2 back)

_CACHE = {}


def _build_nc():
    import concourse.tile as tile
    from concourse import mybir, bacc

    f32 = mybir.dt.float32
    bf16 = mybir.dt.bfloat16

    nc = bacc.Bacc()
    xT = nc.declare_dram_parameter("xT", [BLOC, D, S], bf16, isOutput=False)
    wq = nc.declare_dram_parameter("wq", [D, D], bf16, isOutput=False)
    wk = nc.declare_dram_parameter("wk", [D, D], bf16, isOutput=False)
    wv = nc.declare_dram_parameter("wv", [D, D], bf16, isOutput=False)
    dw = nc.declare_dram_parameter("dw", [D, D], bf16, isOutput=False)
    # compact cluster tables: the dense AE_pair[pr][s=128j+p, c] block j is
    # nonzero only at cols {4j + p//32, 64 + 4j + p//32}; aec[pr, p, j, 0:4]
    # holds cols 4j..4j+3 (head 0), [4:8] cols 64+4j..64+4j+3 (head 1).
    aec = nc.declare_dram_parameter("aec", [NPAIR, 128, NJ, 8], bf16,
                                    isOutput=False)
    afc = nc.declare_dram_parameter("afc", [NPAIR, 128, NJ, 8], bf16,
                                    isOutput=False)
    # per-head tap matrices, duplicated on both partition halves, grouped
    # by K-stack destination: bdma[p, k, c'] = M_{(-2,0,+2)[k]}[p % 64, c']
    # (psum rows 0-63), bdmb[p, k, c'] = M_{(-1,+1)[k]} (psum rows 64-127).
    bdma = nc.declare_dram_parameter("bdma", [128, 3, 64], bf16, isOutput=False)
    bdmb = nc.declare_dram_parameter("bdmb", [128, 2, 64], bf16, isOutput=False)
    onesbd = nc.declare_dram_parameter("onesbd", [128, 128], bf16,
                                       isOutput=False)
    zpad = nc.declare_dram_parameter("zpad", [128, 16 * 136], bf16,
                                     isOutput=False)
    out = nc.declare_dram_parameter("out", [BLOC, S, D], bf16, isOutput=True)

    with tile.TileContext(nc) as tc:
        with tc.tile_pool(name="const", bufs=1) as cpool, \
             tc.tile_pool(name="big", bufs=1) as bigp, \
             tc.tile_pool(name="sm", bufs=4) as smp, \
             tc.tile_pool(name="bd", bufs=8) as bdp, \
             tc.tile_pool(name="ob", bufs=2) as obp, \
             tc.tile_pool(name="psB", bufs=6, space="PSUM") as psB, \
             tc.tile_pool(name="psS", bufs=2, space="PSUM") as psS:

            # ---- constants in SBUF ----
            wq_sb = cpool.tile([128, NDC, D], bf16)
            wk_sb = cpool.tile([128, NDC, D], bf16)
            wv_sb = cpool.tile([128, NDC, D], bf16)
            dw_sb = cpool.tile([128, NDC, D], bf16)
            for t_sb, t_dr in ((wk_sb, wk), (wv_sb, wv), (wq_sb, wq),
                               (dw_sb, dw)):
                nc.sync.dma_start(out=t_sb, in_=t_dr[:].rearrange("(o p) m -> p o m", p=128))
            bdma_sb = cpool.tile([128, 3, 64], bf16)
            nc.sync.dma_start(out=bdma_sb, in_=bdma[:])
            bdmb_sb = cpool.tile([128, 2, 64], bf16)
            nc.sync.dma_start(out=bdmb_sb, in_=bdmb[:])
            ones_sb = cpool.tile([128, 128], bf16)
            nc.sync.dma_start(out=ones_sb, in_=onesbd[:])

            # expanded cluster tables: 8 persistent SBUF tiles (per pair x
            # ae/af, shared by both batches), fetched ONCE at start.  Zero
            # background comes from a DMA'd zeros param; the value writes go
            # through a stride-136 view (block j's values land at flat col
            # 136j + {0..3, 64..67}), reads through a stride-132 view whose
            # block j = flat cols [132j, 132j+128) — within it the values sit
            # at in-block cols 4j+{p//32} and 64+4j+{p//32}, exactly the
            # dense table layout.
            AEW = 16 * 136
            aexp = {}
            for pr in range(NPAIR):
                for tb, src in (("ae", aec), ("af", afc)):
                    t = cpool.tile([128, AEW], bf16, name=f"aexp_{tb}{pr}")
                    aexp[(tb, pr)] = t
                    nc.sync.dma_start(out=t, in_=zpad[:])
                    w = t.rearrange("p (j c) -> p j c", c=136)
                    nc.sync.dma_start(out=w[:, :, 0:4], in_=src[pr, :, :, 0:4])
                    nc.sync.dma_start(out=w[:, :, 64:68],
                                      in_=src[pr, :, :, 4:8])

            st = [dict() for _ in range(BLOC)]

            def emit_x_load(b):
                # column-sliced so kv/qt of early s-chunks start ASAP;
                # dc slices split across two DMA queues to double feed rate.
                s = st[b]
                s["xt"] = [bigp.tile([128, S], bf16, tag="xt", bufs=2 * NDC,
                                     name=f"xt_{b}_{dc}")
                           for dc in range(NDC)]
                for q in range(SCH):
                    for dc in range(NDC):
                        nc.gpsimd.dma_start(
                            out=s["xt"][dc][:, SCW * q:SCW * (q + 1)],
                            in_=xT[b, 128 * dc:128 * (dc + 1),
                                   SCW * q:SCW * (q + 1)])

            def emit_kv(b, j):
                s = st[b]
                if j == 0:
                    s["knat"] = bigp.tile([128, NJ, D], bf16, tag="knat",
                                          name=f"knat_{b}")
                    s["vnat"] = bigp.tile([128, NJ, D], bf16, tag="vnat",
                                          name=f"vnat_{b}")
                for w_sb, key in ((wk_sb, "knat"), (wv_sb, "vnat")):
                    ps_k = psB.tile([128, D], f32, tag="ps512")
                    for dc in range(NDC):
                        nc.tensor.matmul(
                            ps_k,
                            s["xt"][dc][:, 128 * j:128 * (j + 1)],
                            w_sb[:, dc, :],
                            start=(dc == 0), stop=(dc == NDC - 1))
                    if key == "knat":
                        nc.vector.tensor_copy(out=s[key][:, j, :], in_=ps_k)
                    else:
                        nc.scalar.copy(out=s[key][:, j, :], in_=ps_k)

            def emit_qt(b, pr, n):
                # per-pair duplicated qT tile [128, 2 heads, QW]: partitions
                # 0-63 hold q[d, col-2], partitions 64-127 q[d, col-1].
                # b1's pair tiles ring-reuse b0's as those pairs retire.
                s = st[b]
                if n == 0:
                    t = bigp.tile([128, 2, QW], bf16, tag="qtsp", bufs=5,
                                  name=f"qts_{b}_{pr}")
                    s.setdefault("qtsp", {})[pr] = t
                    nc.vector.memset(t[0:64, :, 0:2], 0.0)
                    nc.vector.memset(t[0:64, :, S + 2:], 0.0)
                    nc.vector.memset(t[64:128, :, 0:1], 0.0)
                    nc.vector.memset(t[64:128, :, S + 1:], 0.0)
                ps_q = psB.tile([128, SCW], f32, tag="ps512")
                for dc in range(NDC):
                    nc.tensor.matmul(
                        ps_q,
                        wq_sb[:, dc, 128 * pr:128 * (pr + 1)],
                        s["xt"][dc][:, SCW * n:SCW * (n + 1)],
                        start=(dc == 0), stop=(dc == NDC - 1))
                qts = s["qtsp"][pr]
                b0 = 2 + SCW * n       # block0 col of s = SCW*n
                # one full-width ACT copy: h0 rows into slot 0 block0;
                # h1 rows land in slot 0's upper half (staging).
                nc.scalar.copy(out=qts[:, 0, b0:b0 + SCW], in_=ps_q)

            def emit_qdup(b, pr):
                # after all 4 chunks of this pair are staged, build the
                # duplicated layouts with 3 full-width SBUF->SBUF DMAs
                # (same queue => in order; gpsimd queue, behind the x feed).
                s = st[b]
                qts = s["qtsp"][pr]
                #  a) h1 block1 <- staged h1 rows (same partitions, col -1)
                nc.gpsimd.dma_start(out=qts[64:128, 1, 1:1 + S],
                                    in_=qts[64:128, 0, 2:2 + S])
                #  b) h1 block0 <- staged h1 rows (cross partition 64->0)
                nc.gpsimd.dma_start(out=qts[0:64, 1, 2:2 + S],
                                    in_=qts[64:128, 0, 2:2 + S])
                #  c) h0 block1 <- h0 block0 (cross 0->64, col -1); clobbers
                #     the staging except its last column...
                nc.gpsimd.dma_start(out=qts[64:128, 0, 1:1 + S],
                                    in_=qts[0:64, 0, 2:2 + S])
                #     ... and its last column must read q(S)=0: copy the
                #     always-zero block1 col 0 (same queue => after a/b).
                nc.gpsimd.dma_start(out=qts[64:128, 0, S + 1:S + 2],
                                    in_=qts[64:128, 0, 0:1])

            def emit_proj_fetch(b, pr):
                pass  # tables are persistent; fetched once at start

            def emit_proj(b, pr):
                s = st[b]
                if pr == 0:
                    s["kp"] = bigp.tile([128, NPAIR, 128], bf16, tag="kpbd",
                                        bufs=2, name=f"kp_{b}")
                    s["vp"] = bigp.tile([128, NPAIR, 128], bf16, tag="vpbd",
                                        bufs=2, name=f"vp_{b}")
                    nc.vector.memset(s["vp"], 0.0)
                for a_sb, key, dstk in ((aexp[("ae", pr)], "knat", "kp"),
                                        (aexp[("af", pr)], "vnat", "vp")):
                    a_rd = a_sb[:, 0:16 * 132].rearrange(
                        "p (j c) -> p j c", c=132)
                    ps_p = psS.tile([128, 128], f32, tag="pssmall")
                    for j in range(NJ):
                        nc.tensor.matmul(
                            ps_p,
                            a_rd[:, j, 0:128],
                            st[b][key][:, j, 128 * pr:128 * (pr + 1)],
                            start=(j == 0), stop=(j == NJ - 1))
                    dst = st[b][dstk]
                    if dstk == "kp":
                        # only diag blocks are ever read (per-head lhsT)
                        nc.vector.tensor_copy(out=dst[:, pr, :], in_=ps_p)
                    else:
                        # vp is used as a block-diag [c,d] operand: keep
                        # off-diag zero.
                        nc.vector.tensor_copy(
                            out=dst[0:64, pr, 0:64], in_=ps_p[0:64, 0:64])
                        nc.vector.tensor_copy(
                            out=dst[64:128, pr, 64:128],
                            in_=ps_p[64:128, 64:128])

            def emit_kt(b, pr):
                # per-head K-stacked tap operands:
                # T[:,0]=[bdt(-2);bdt(-1)], T[:,1]=[bdt(0);bdt(+1)],
                # T[0:64,2]=bdt(+2), where bdt(t) = kp_h^T @ M_t  [d, c'].
                s = st[b]
                if pr == 0:
                    s["bdts"] = {}
                    s["expt"] = {}
                    s["cw"] = {}
                for h2 in (0, 1):
                    hb = 64 * h2
                    h = 2 * pr + h2
                    kp_h = s["kp"][hb:hb + 64, pr, hb:hb + 64]
                    T = bdp.tile([128, 3, 64], bf16, tag="bdts",
                                 name=f"bdts_{b}_{h}")
                    s["bdts"][h] = T
                    ps_b = psS.tile([128, 3, 64], f32, tag="pssmall")
                    nc.tensor.matmul(ps_b[0:64, :, :], kp_h,
                                     bdma_sb[hb:hb + 64, :, :],
                                     start=True, stop=True)
                    nc.tensor.matmul(ps_b[64:128, 0:2, :], kp_h,
                                     bdmb_sb[hb:hb + 64, :, :],
                                     start=True, stop=True)
                    nc.scalar.copy(out=T[:, 0:2, :], in_=ps_b[:, 0:2, :])
                    nc.scalar.copy(out=T[0:64, 2, :], in_=ps_b[0:64, 2, :])

            def emit_scores(b, pr, n):
                # 3 K-stacked tap matmuls per head; heads in different PE
                # column groups so consecutive pairs overlap.
                s = st[b]
                qts = s["qtsp"][pr]
                ps_sc = psB.tile([128, SCW], f32, tag="ps512")
                base = SCW * n
                for h2 in (0, 1):
                    hb = 64 * h2
                    h = 2 * pr + h2
                    T = s["bdts"][h]
                    nc.tensor.matmul(ps_sc[hb:hb + 64, :], T[:, 0, :],
                                     qts[:, h2, base:base + SCW],
                                     start=True, stop=False)
                for h2 in (0, 1):
                    hb = 64 * h2
                    h = 2 * pr + h2
                    T = s["bdts"][h]
                    nc.tensor.matmul(ps_sc[hb:hb + 64, :], T[:, 1, :],
                                     qts[:, h2, base + 2:base + 2 + SCW],
                                     start=False, stop=False)
                for h2 in (0, 1):
                    hb = 64 * h2
                    h = 2 * pr + h2
                    T = s["bdts"][h]
                    nc.tensor.matmul(ps_sc[hb:hb + 64, :], T[0:64, 2, :],
                                     qts[0:64, h2, base + 4:base + 4 + SCW],
                                     start=False, stop=True)
                expt = smp.tile([128, SCW], bf16, tag="expt", bufs=4)
                nc.scalar.activation(
                    out=expt, in_=ps_sc,
                    func=mybir.ActivationFunctionType.Exp)
                s["expt"][(pr, n)] = expt

            def emit_zat(b, pr, n):
                s = st[b]
                expt = s["expt"].pop((pr, n))
                if pr == 0:
                    s["cw"][n] = bigp.tile([128, NPAIR, SCW], bf16,
                                           tag="cwin", bufs=6,
                                           name=f"cw_{b}_{n}")
                ps_z = psB.tile([128, SCW], f32, tag="ps512")
                nc.tensor.matmul(ps_z, ones_sb, expt, start=True, stop=True)
                ps_at = psB.tile([128, SCW], f32, tag="ps512")
                nc.tensor.matmul(ps_at, s["vp"][:, pr, :], expt,
                                 start=True, stop=True)
                rzb = smp.tile([128, SCW], f32, tag="rzb", bufs=1)
                nc.vector.reciprocal_approx_fast(out=rzb, in_=ps_z)
                nc.vector.tensor_mul(
                    out=s["cw"][n][:, pr, :], in0=ps_at, in1=rzb)

            def emit_dense(b, j):
                s = st[b]
                n, jj = j // 4, j % 4
                cw = s["cw"][n]
                ps_d = psB.tile([128, D], f32, tag="ps512")
                for dc in range(NDC):
                    nc.tensor.matmul(
                        ps_d,
                        cw[:, dc, 128 * jj:128 * (jj + 1)],
                        dw_sb[:, dc, :],
                        start=(dc == 0), stop=(dc == NDC - 1))
                emit_dense_out(b, j, ps_d)

            def emit_dense_out(b, j, ps_d):
                obuf = obp.tile([128, D], bf16, tag="obuf")
                if j % 2:
                    nc.vector.tensor_copy(out=obuf, in_=ps_d)
                else:
                    nc.scalar.copy(out=obuf, in_=ps_d)
                eng = (nc.sync, nc.gpsimd, nc.scalar)[j % 3]
                eng.dma_start(out=out[b, 128 * j:128 * (j + 1), :], in_=obuf)

            def emit_dense_partial(b, j, dcs, ps_d):
                # progressive tail dense: accumulate listed dc chunks of
                # output tile j; finish (copy+DMA) when dc 3 lands.
                s = st[b]
                n, jj = j // 4, j % 4
                cw = s["cw"][n]
                for dc in dcs:
                    nc.tensor.matmul(
                        ps_d,
                        cw[:, dc, 128 * jj:128 * (jj + 1)],
                        dw_sb[:, dc, :],
                        start=(dc == 0), stop=(dc == NDC - 1))
                if dcs[-1] == NDC - 1:
                    emit_dense_out(b, j, ps_d)

            # ================= emission schedule =================
            from collections import deque

            # Phase A: batch-0 GEMMs per x-slice quarter; batch-1 x DMAs
            # queued right behind batch-0's.
            emit_x_load(0)
            emit_x_load(1)
            for q in range(SCH):
                for j in range(4 * q, 4 * q + 4):
                    emit_kv(0, j)
                for pr in range(NPAIR):
                    emit_qt(0, pr, q)
                if q == 2:
                    emit_proj_fetch(0, 0)
                    emit_proj_fetch(0, 1)
            for pr in range(NPAIR):
                emit_qdup(0, pr)
            emit_proj(0, 0)
            emit_proj_fetch(0, 2)
            emit_qt(1, 0, 0)
            emit_kt(0, 0)
            emit_proj(0, 1)
            emit_proj_fetch(0, 3)
            emit_qt(1, 0, 1)
            emit_kt(0, 1)
            emit_proj(0, 2)
            emit_qt(1, 0, 2)
            emit_kt(0, 2)
            emit_proj(0, 3)
            emit_qt(1, 0, 3)
            emit_kt(0, 3)
            emit_qdup(1, 0)

            # Phase B: batch-0 attention with batch-1 GEMM units as fillers.
            qt_units = []
            for pr in range(1, NPAIR):
                qt_units += [(emit_qt, (1, pr, n)) for n in range(SCH)]
                qt_units.append((emit_qdup, (1, pr)))
            fill = deque(
                [(emit_kv, (1, j)) for j in range(8)] +
                [(emit_proj_fetch, (1, 0))] +
                [(emit_kv, (1, j)) for j in range(8, NJ)] +
                [(emit_proj_fetch, (1, 1))] +
                qt_units +
                [(emit_proj, (1, 0)), (emit_proj_fetch, (1, 2)),
                 (emit_kt, (1, 0)),
                 (emit_proj, (1, 1)), (emit_proj_fetch, (1, 3)),
                 (emit_kt, (1, 1)),
                 (emit_proj, (1, 2)), (emit_kt, (1, 2)),
                 (emit_proj, (1, 3)), (emit_kt, (1, 3))])

            def popf():
                # emit filler units until one with PE work was emitted
                while fill:
                    f, a = fill.popleft()
                    f(*a)
                    if f not in (emit_proj_fetch, emit_qdup):
                        break

            # scores runs one unit ahead of zat so the ACT exp latency is
            # always covered by the next unit's matmuls even with no fillers.
            prev = None
            for pr in range(NPAIR):
                for n in range(SCH):
                    emit_scores(0, pr, n)
                    popf()
                    if prev is not None:
                        emit_zat(0, *prev)
                        popf()
                    prev = (pr, n)
            emit_zat(0, *prev)
            while fill:
                popf()

            # Phase C: batch-1 attention (scores pipelined one ahead of zat);
            # fillers are batch-0 dense then batch-1 dense as chunks complete.
            # The last chunk's dense is accumulated progressively per pair so
            # only the dc=3 matmuls remain after the final zat.
            fill = deque([(emit_dense, (0, j)) for j in range(NJ)])
            prev = None
            ps_tail = None
            for n in range(SCH):
                for pr in range(NPAIR):
                    emit_scores(1, pr, n)
                    popf()
                    if prev is not None:
                        emit_zat(1, *prev)
                        popf()
                        if prev[1] == SCH - 1 and prev[0] == 2:
                            # pairs 0-2 of the last chunk are done: run their
                            # dense contributions now (dc = pair index).
                            ps_tail = [psB.tile([128, D], f32, tag="ps512",
                                                name=f"ps_tail_{jj}")
                                       for jj in range(4)]
                            for jj, ps_d in enumerate(ps_tail):
                                emit_dense_partial(1, 12 + jj, (0, 1, 2), ps_d)
                    prev = (pr, n)
                if n < SCH - 1:
                    for j in range(4 * n, 4 * n + 4):
                        fill.append((emit_dense, (1, j)))
            emit_zat(1, *prev)
            for jj, ps_d in enumerate(ps_tail):
                emit_dense_partial(1, 12 + jj, (3,), ps_d)
            while fill:
                popf()

    nc.finalize()
    return nc


def _prep_inputs(x, mask, wq, wk, wv, EW, FW, conv_w1, conv_w3, conv_w5, conv_b,
                 dense_w, dense_b, cluster_table):
    """Host-side restructuring -> per-core input maps."""
    bf = ml_dtypes.bfloat16
    x = np.ascontiguousarray(np.asarray(x, np.float32))
    mask = np.asarray(mask)
    counts = np.clip(mask.astype(np.int64).sum(1), 1, S)
    pos = np.asarray(cluster_table)[counts - 1]          # [B, P, C]
    if not (pos == pos[0]).all():
        raise NotImplementedError("per-batch cluster tables not supported")
    p0 = pos[0]                                          # [P, C]

    scale = 1.0 / np.sqrt(np.float32(DEPTH))
    s_idx = p0.ravel()
    c_idx = np.repeat(np.arange(P), C)

    def build_table(W, sc):
        A = np.zeros((H, S + 1, P), np.float32)
        np.add.at(A, (np.arange(H)[:, None], s_idx[None, :], c_idx[None, :]),
                  np.asarray(W, np.float32).reshape(H, P * C) * sc)
        return np.ascontiguousarray(A[:, :S, :])

    AE = build_table(EW, scale)
    AF = build_table(FW, 1.0)
    # pack adjacent heads side by side: [NPAIR, S, 128]
    AE = np.ascontiguousarray(
        AE.reshape(NPAIR, 2, S, P).transpose(0, 2, 1, 3).reshape(NPAIR, S, 128))
    AF = np.ascontiguousarray(
        AF.reshape(NPAIR, 2, S, P).transpose(0, 2, 1, 3).reshape(NPAIR, S, 128))
    # partition-major for fast DMA: [NPAIR, 128, NJ, 128]
    AE = np.ascontiguousarray(
        AE.reshape(NPAIR, NJ, 128, 128).transpose(0, 2, 1, 3))
    AF = np.ascontiguousarray(
        AF.reshape(NPAIR, NJ, 128, 128).transpose(0, 2, 1, 3))

    def compact(A):
        # keep only block j's cols {4j..4j+3, 64+4j..64+4j+3}; valid for the
        # contiguous 32-wide clusters of a full mask.
        out = np.zeros((NPAIR, 128, NJ, 8), np.float32)
        chk = np.zeros_like(A)
        for j in range(NJ):
            out[:, :, j, 0:4] = A[:, :, j, 4 * j:4 * j + 4]
            out[:, :, j, 4:8] = A[:, :, j, 64 + 4 * j:64 + 4 * j + 4]
            chk[:, :, j, 4 * j:4 * j + 4] = out[:, :, j, 0:4]
            chk[:, :, j, 64 + 4 * j:64 + 4 * j + 4] = out[:, :, j, 4:8]
        if not np.array_equal(chk, A):
            raise NotImplementedError("non-contiguous clusters")
        return out

    AEC, AFC = compact(AE), compact(AF)

    # conv -> 5 tap matrices (per-head [P, P], duplicated on both halves)
    wp = np.arange(P)[:, None]
    jj = np.arange(P)[None, :]
    ii = wp - jj + 31
    valid = (ii >= 0) & (ii < P)
    ii = np.clip(ii, 0, P - 1)
    M = {t: np.zeros((P, P), np.float32) for t in range(-2, 3)}
    for cw, hk in ((conv_w1, 1), (conv_w3, 3), (conv_w5, 5)):
        cw = np.asarray(cw, np.float32)
        pad = (hk - 1) // 2
        for dy in range(hk):
            filt = cw[dy, :, 0, 0]
            M[dy - pad] += np.where(valid, filt[ii], 0.0) / 3.0
    BDMA = np.zeros((128, 3, P), np.float32)
    for k, t in enumerate((-2, 0, 2)):
        BDMA[:64, k, :] = M[t]
        BDMA[64:, k, :] = M[t]
    BDMB = np.zeros((128, 2, P), np.float32)
    for k, t in enumerate((-1, 1)):
        BDMB[:64, k, :] = M[t]
        BDMB[64:, k, :] = M[t]
    bbar = float(np.asarray(conv_b, np.float32).mean())
    if abs(bbar) > 1e-30:
        raise NotImplementedError("nonzero conv bias not folded")

    ones_bd = np.zeros((128, 128), np.float32)
    ones_bd[:64, :64] = 1.0
    ones_bd[64:, 64:] = 1.0

    # shard + transpose x
    xsh = x.reshape(NCORES, BLOC, S, D)
    in_maps = []
    shared = dict(
        wq=np.asarray(wq, np.float32).astype(bf),
        wk=np.asarray(wk, np.float32).astype(bf),
        wv=np.asarray(wv, np.float32).astype(bf),
        dw=np.asarray(dense_w, np.float32).astype(bf),
        aec=AEC.astype(bf), afc=AFC.astype(bf),
        zpad=np.zeros((128, 16 * 136), bf),
        bdma=BDMA.astype(bf), bdmb=BDMB.astype(bf),
        onesbd=ones_bd.astype(bf),
    )
    for c in range(NCORES):
        m = dict(shared)
        m["xT"] = np.ascontiguousarray(xsh[c].transpose(0, 2, 1)).astype(bf)
        in_maps.append(m)
    return in_maps


def _run(in_maps, trace=False, tmpdir=None):
    from concourse.bass_utils import run_bass_kernel_spmd
    if "nc" not in _CACHE:
        _CACHE["nc"] = _build_nc()
    kw = {}
    if trace:
        _install_ntff_hook()
        kw = dict(trace=True, tmpdir=tmpdir)
    return run_bass_kernel_spmd(_CACHE["nc"], in_maps,
                                core_ids=list(range(NCORES)), **kw)


def _install_ntff_hook():
    import types, importlib.util as ilu
    if "antenv.axon_hooks" in sys.modules:
        return
    spec = ilu.spec_from_file_location(
        "trn_boot_mod", "/root/.axon_site/trn_agent_boot/trn_boot.py")
    tb = ilu.module_from_spec(spec)
    spec.loader.exec_module(tb)
    hook = tb._ntff_profile_via_ctypes("/opt/axon/libaxon_pjrt.so")
    mod = types.ModuleType("antenv.axon_hooks")
    mod.get_axon_ntff_profile_hook = lambda: hook
    import antenv  # noqa: F401
    sys.modules["antenv.axon_hooks"] = mod


def kernel(**inputs) -> np.ndarray:
    in_maps = _prep_inputs(**inputs)
    r = _run(in_maps)
    out = np.concatenate([np.asarray(r.results[c]["out"], np.float32)
                          for c in range(NCORES)], axis=0)
    db = np.asarray(inputs["dense_b"], np.float32)
    if np.any(db):  # dense bias applied host-side (zero in practice)
        out = out + db
    return out



# revision 34
# speedup vs baseline: 1.1367x; 1.0612x over previous
"""Clustered Linformer Attention — Trainium2 Bass kernel, 8 NeuronCores.

Strategy: data-parallel over batch (2 batches/core, no collectives).
Math restructuring (verified vs reference to ~7e-7 in f32):
  - mask is all-ones => cluster c holds positions [32c, 32c+32); the per-head
    gather+einsum projections become  k_proj = AE[h]^T @ k_h  with a host-built
    sparse table AE[h] in [S, P] (score scale folded in), same for v with AF.
  - the 3-kernel conv fusion over scores collapses to 5 "tap" matrices M_t in
    [P, P] (t in -2..2):  scores_conv[s] = sum_t  (q[s+t] @ (k_proj^T @ M_t)).
  - v2: the 5 taps are K-STACKED two-per-matmul: qts stores each head's qT
    twice (partitions 0-63 at shift 0, 64-127 at shift +1), and the tap
    operands T0=[bdt(-2);bdt(-1)], T1=[bdt(0);bdt(+1)], T2=bdt(+2) contract
    over 128/128/64 partitions.  The two heads of a pair run in different PE
    column groups (tile_position col 0 / 64), so the 3 matmuls per head
    overlap pairwise -> ~3 matmul-times for what used to take 5.
  - softmax has no max-subtraction (|scores| <~ 1.6, exp is safe in f32);
    Z = sum_c exp is computed by an all-ones block-diag matmul that also
    broadcasts Z to all 128 partitions, so normalization is one DVE op.

Scheduling: x is DMA'd in column slices so QKV starts early; ae/af cluster
tables are partition-major in DRAM (4KB rows) and prefetched per-pair; each
attention unit is split into scores(+exp) and Z/at(+normalize) halves with a
filler matmul unit between them in PE program order (covers ACT exp latency);
dense output copies alternate ACT/DVE and output DMAs alternate queues.
"""
import sys
import numpy as np
import ml_dtypes

sys.path.insert(0, '/opt/trn_rl_repo')

B, S, D = 16, 2048, 512
H, P, C = 8, 64, 32
DEPTH = D // H           # 64
NCORES = 8
BLOC = B // NCORES       # 2 batches per core
NPAIR = H // 2           # 4 head pairs
SCH = 4                  # s-chunks of 512
SCW = S // SCH           # 512
NJ = S // 128            # 16 s-tiles of 128
NDC = D // 128           # 4 contraction chunks
QW = S + 4               # qts width (2 pad front, 2 back)

_CACHE = {}


def _build_nc():
    import concourse.tile as tile
    from concourse import mybir, bacc

    f32 = mybir.dt.float32
    bf16 = mybir.dt.bfloat16

    nc = bacc.Bacc()
    xT = nc.declare_dram_parameter("xT", [BLOC, D, S], bf16, isOutput=False)
    wq = nc.declare_dram_parameter("wq", [D, D], bf16, isOutput=False)
    wk = nc.declare_dram_parameter("wk", [D, D], bf16, isOutput=False)
    wv = nc.declare_dram_parameter("wv", [D, D], bf16, isOutput=False)
    dw = nc.declare_dram_parameter("dw", [D, D], bf16, isOutput=False)
    # partition-major cluster tables: ae[pr, p, j, c] = AE_pair[pr][128j+p, c]
    ae = nc.declare_dram_parameter("ae", [NPAIR, 128, NJ, 128], bf16,
                                   isOutput=False)
    af = nc.declare_dram_parameter("af", [NPAIR, 128, NJ, 128], bf16,
                                   isOutput=False)
    # per-head tap matrices, duplicated on both partition halves, grouped
    # by K-stack destination: bdma[p, k, c'] = M_{(-2,0,+2)[k]}[p % 64, c']
    # (psum rows 0-63), bdmb[p, k, c'] = M_{(-1,+1)[k]} (psum rows 64-127).
    bdma = nc.declare_dram_parameter("bdma", [128, 3, 64], bf16, isOutput=False)
    bdmb = nc.declare_dram_parameter("bdmb", [128, 2, 64], bf16, isOutput=False)
    onesbd = nc.declare_dram_parameter("onesbd", [128, 128], bf16,
                                       isOutput=False)
    out = nc.declare_dram_parameter("out", [BLOC, S, D], bf16, isOutput=True)

    with tile.TileContext(nc) as tc:
        with tc.tile_pool(name="const", bufs=1) as cpool, \
             tc.tile_pool(name="big", bufs=1) as bigp, \
             tc.tile_pool(name="sm", bufs=4) as smp, \
             tc.tile_pool(name="bd", bufs=8) as bdp, \
             tc.tile_pool(name="ob", bufs=2) as obp, \
             tc.tile_pool(name="psB", bufs=6, space="PSUM") as psB, \
             tc.tile_pool(name="psS", bufs=2, space="PSUM") as psS:

            # ---- constants in SBUF ----
            wq_sb = cpool.tile([128, NDC, D], bf16)
            wk_sb = cpool.tile([128, NDC, D], bf16)
            wv_sb = cpool.tile([128, NDC, D], bf16)
            dw_sb = cpool.tile([128, NDC, D], bf16)
            for t_sb, t_dr in ((wk_sb, wk), (wv_sb, wv), (wq_sb, wq),
                               (dw_sb, dw)):
                nc.sync.dma_start(out=t_sb, in_=t_dr[:].rearrange("(o p) m -> p o m", p=128))
            bdma_sb = cpool.tile([128, 3, 64], bf16)
            nc.sync.dma_start(out=bdma_sb, in_=bdma[:])
            bdmb_sb = cpool.tile([128, 2, 64], bf16)
            nc.sync.dma_start(out=bdmb_sb, in_=bdmb[:])
            ones_sb = cpool.tile([128, 128], bf16)
            nc.sync.dma_start(out=ones_sb, in_=onesbd[:])

            # cluster tables: 8 persistent SBUF tiles (per pair x ae/af,
            # shared by both batches), each fetched ONCE at start with a
            # single contiguous DMA on the sync queue.
            aexp = {}
            for pr in range(NPAIR):
                for tb, srct in (("ae", ae), ("af", af)):
                    t = cpool.tile([128, NJ, 128], bf16, name=f"aexp_{tb}{pr}")
                    aexp[(tb, pr)] = t
                    nc.sync.dma_start(out=t, in_=srct[pr])

            st = [dict() for _ in range(BLOC)]

            def emit_x_load(b):
                # column-sliced so kv/qt of early s-chunks start ASAP.
                # batch 0 feeds phase A from the gpsimd queue; batch 1 rides
                # the sync queue (behind weights+tables, done well before
                # phase B needs it) so the two batches transfer in parallel.
                s = st[b]
                s["xt"] = [bigp.tile([128, S], bf16, tag="xt", bufs=2 * NDC,
                                     name=f"xt_{b}_{dc}")
                           for dc in range(NDC)]
                eng = nc.gpsimd if b == 0 else nc.sync
                for q in range(SCH):
                    for dc in range(NDC):
                        eng.dma_start(
                            out=s["xt"][dc][:, SCW * q:SCW * (q + 1)],
                            in_=xT[b, 128 * dc:128 * (dc + 1),
                                   SCW * q:SCW * (q + 1)])

            def emit_kv(b, j):
                s = st[b]
                if j == 0:
                    s["knat"] = bigp.tile([128, NJ, D], bf16, tag="knat",
                                          name=f"knat_{b}")
                    s["vnat"] = bigp.tile([128, NJ, D], bf16, tag="vnat",
                                          name=f"vnat_{b}")
                for w_sb, key in ((wk_sb, "knat"), (wv_sb, "vnat")):
                    ps_k = psB.tile([128, D], f32, tag="ps512")
                    for dc in range(NDC):
                        nc.tensor.matmul(
                            ps_k,
                            s["xt"][dc][:, 128 * j:128 * (j + 1)],
                            w_sb[:, dc, :],
                            start=(dc == 0), stop=(dc == NDC - 1))
                    if key == "knat":
                        nc.vector.tensor_copy(out=s[key][:, j, :], in_=ps_k)
                    else:
                        nc.scalar.copy(out=s[key][:, j, :], in_=ps_k)

            def emit_qt(b, pr, n):
                # per-pair duplicated qT tile [128, 2 heads, QW]: partitions
                # 0-63 hold q[d, col-2], partitions 64-127 q[d, col-1].
                # b1's pair tiles ring-reuse b0's as those pairs retire.
                s = st[b]
                if n == 0:
                    t = bigp.tile([128, 2, QW], bf16, tag="qtsp", bufs=5,
                                  name=f"qts_{b}_{pr}")
                    s.setdefault("qtsp", {})[pr] = t
                    nc.vector.memset(t[0:64, :, 0:2], 0.0)
                    nc.vector.memset(t[0:64, :, S + 2:], 0.0)
                    nc.vector.memset(t[64:128, :, 0:1], 0.0)
                    nc.vector.memset(t[64:128, :, S + 1:], 0.0)
                ps_q = psB.tile([128, SCW], f32, tag="ps512")
                for dc in range(NDC):
                    nc.tensor.matmul(
                        ps_q,
                        wq_sb[:, dc, 128 * pr:128 * (pr + 1)],
                        s["xt"][dc][:, SCW * n:SCW * (n + 1)],
                        start=(dc == 0), stop=(dc == NDC - 1))
                qts = s["qtsp"][pr]
                b0 = 2 + SCW * n       # block0 col of s = SCW*n
                # one full-width ACT copy: h0 rows into slot 0 block0;
                # h1 rows land in slot 0's upper half (staging).
                nc.scalar.copy(out=qts[:, 0, b0:b0 + SCW], in_=ps_q)

            def emit_qdup(b, pr):
                # after all 4 chunks of this pair are staged, build the
                # duplicated layouts with 3 full-width SBUF->SBUF DMAs
                # (same queue => in order; gpsimd queue, behind the x feed).
                s = st[b]
                qts = s["qtsp"][pr]
                #  a) h1 block1 <- staged h1 rows (same partitions, col -1)
                nc.gpsimd.dma_start(out=qts[64:128, 1, 1:1 + S],
                                    in_=qts[64:128, 0, 2:2 + S])
                #  b) h1 block0 <- staged h1 rows (cross partition 64->0)
                nc.gpsimd.dma_start(out=qts[0:64, 1, 2:2 + S],
                                    in_=qts[64:128, 0, 2:2 + S])
                #  c) h0 block1 <- h0 block0 (cross 0->64, col -1); clobbers
                #     the staging except its last column...
                nc.gpsimd.dma_start(out=qts[64:128, 0, 1:1 + S],
                                    in_=qts[0:64, 0, 2:2 + S])
                #     ... and its last column must read q(S)=0: copy the
                #     always-zero block1 col 0 (same queue => after a/b).
                nc.gpsimd.dma_start(out=qts[64:128, 0, S + 1:S + 2],
                                    in_=qts[64:128, 0, 0:1])

            def emit_proj_fetch(b, pr):
                pass  # tables are persistent; fetched once at start

            def emit_proj(b, pr):
                s = st[b]
                if pr == 0:
                    s["kp"] = bigp.tile([128, NPAIR, 128], bf16, tag="kpbd",
                                        bufs=2, name=f"kp_{b}")
                    s["vp"] = bigp.tile([128, NPAIR, 128], bf16, tag="vpbd",
                                        bufs=2, name=f"vp_{b}")
                    nc.vector.memset(s["vp"], 0.0)
                for a_sb, key, dstk in ((aexp[("ae", pr)], "knat", "kp"),
                                        (aexp[("af", pr)], "vnat", "vp")):
                    ps_p = psS.tile([128, 128], f32, tag="pssmall")
                    for j in range(NJ):
                        nc.tensor.matmul(
                            ps_p,
                            a_sb[:, j, :],
                            st[b][key][:, j, 128 * pr:128 * (pr + 1)],
                            start=(j == 0), stop=(j == NJ - 1))
                    dst = st[b][dstk]
                    if dstk == "kp":
                        # only diag blocks are ever read (per-head lhsT)
                        nc.vector.tensor_copy(out=dst[:, pr, :], in_=ps_p)
                    else:
                        # vp is used as a block-diag [c,d] operand: keep
                        # off-diag zero.
                        nc.vector.tensor_copy(
                            out=dst[0:64, pr, 0:64], in_=ps_p[0:64, 0:64])
                        nc.vector.tensor_copy(
                            out=dst[64:128, pr, 64:128],
                            in_=ps_p[64:128, 64:128])

            def emit_kt(b, pr):
                # per-head K-stacked tap operands:
                # T[:,0]=[bdt(-2);bdt(-1)], T[:,1]=[bdt(0);bdt(+1)],
                # T[0:64,2]=bdt(+2), where bdt(t) = kp_h^T @ M_t  [d, c'].
                s = st[b]
                if pr == 0:
                    s["bdts"] = {}
                    s["expt"] = {}
                    s["cw"] = {}
                for h2 in (0, 1):
                    hb = 64 * h2
                    h = 2 * pr + h2
                    kp_h = s["kp"][hb:hb + 64, pr, hb:hb + 64]
                    T = bdp.tile([128, 3, 64], bf16, tag="bdts",
                                 name=f"bdts_{b}_{h}")
                    s["bdts"][h] = T
                    ps_b = psS.tile([128, 3, 64], f32, tag="pssmall")
                    nc.tensor.matmul(ps_b[0:64, :, :], kp_h,
                                     bdma_sb[hb:hb + 64, :, :],
                                     start=True, stop=True)
                    nc.tensor.matmul(ps_b[64:128, 0:2, :], kp_h,
                                     bdmb_sb[hb:hb + 64, :, :],
                                     start=True, stop=True)
                    nc.scalar.copy(out=T[:, 0:2, :], in_=ps_b[:, 0:2, :])
                    nc.scalar.copy(out=T[0:64, 2, :], in_=ps_b[0:64, 2, :])

            def emit_scores(b, pr, n):
                # 3 K-stacked tap matmuls per head; heads in different PE
                # column groups so consecutive pairs overlap.
                s = st[b]
                qts = s["qtsp"][pr]
                ps_sc = psB.tile([128, SCW], f32, tag="ps512")
                base = SCW * n
                for h2 in (0, 1):
                    hb = 64 * h2
                    h = 2 * pr + h2
                    T = s["bdts"][h]
                    nc.tensor.matmul(ps_sc[hb:hb + 64, :], T[:, 0, :],
                                     qts[:, h2, base:base + SCW],
                                     start=True, stop=False)
                for h2 in (0, 1):
                    hb = 64 * h2
                    h = 2 * pr + h2
                    T = s["bdts"][h]
                    nc.tensor.matmul(ps_sc[hb:hb + 64, :], T[:, 1, :],
                                     qts[:, h2, base + 2:base + 2 + SCW],
                                     start=False, stop=False)
                for h2 in (0, 1):
                    hb = 64 * h2
                    h = 2 * pr + h2
                    T = s["bdts"][h]
                    nc.tensor.matmul(ps_sc[hb:hb + 64, :], T[0:64, 2, :],
                                     qts[0:64, h2, base + 4:base + 4 + SCW],
                                     start=False, stop=True)
                expt = smp.tile([128, SCW], bf16, tag="expt", bufs=4)
                nc.scalar.activation(
                    out=expt, in_=ps_sc,
                    func=mybir.ActivationFunctionType.Exp)
                s["expt"][(pr, n)] = expt

            def emit_zat(b, pr, n):
                s = st[b]
                expt = s["expt"].pop((pr, n))
                if pr == 0:
                    s["cw"][n] = bigp.tile([128, NPAIR, SCW], bf16,
                                           tag="cwin", bufs=6,
                                           name=f"cw_{b}_{n}")
                ps_z = psB.tile([128, SCW], f32, tag="ps512")
                nc.tensor.matmul(ps_z, ones_sb, expt, start=True, stop=True)
                ps_at = psB.tile([128, SCW], f32, tag="ps512")
                nc.tensor.matmul(ps_at, s["vp"][:, pr, :], expt,
                                 start=True, stop=True)
                rzb = smp.tile([128, SCW], f32, tag="rzb", bufs=1)
                nc.vector.reciprocal_approx_fast(out=rzb, in_=ps_z)
                nc.vector.tensor_mul(
                    out=s["cw"][n][:, pr, :], in0=ps_at, in1=rzb)

            def emit_dense(b, j):
                s = st[b]
                n, jj = j // 4, j % 4
                cw = s["cw"][n]
                ps_d = psB.tile([128, D], f32, tag="ps512")
                for dc in range(NDC):
                    nc.tensor.matmul(
                        ps_d,
                        cw[:, dc, 128 * jj:128 * (jj + 1)],
                        dw_sb[:, dc, :],
                        start=(dc == 0), stop=(dc == NDC - 1))
                emit_dense_out(b, j, ps_d)

            def emit_dense_out(b, j, ps_d):
                obuf = obp.tile([128, D], bf16, tag="obuf", bufs=4)
                if j % 2:
                    nc.vector.tensor_copy(out=obuf, in_=ps_d)
                else:
                    nc.scalar.copy(out=obuf, in_=ps_d)
                eng = (nc.sync, nc.scalar)[j % 2]
                eng.dma_start(out=out[b, 128 * j:128 * (j + 1), :], in_=obuf)

            def emit_dense_partial(b, j, dcs, ps_d):
                # progressive tail dense: accumulate listed dc chunks of
                # output tile j; finish (copy+DMA) when dc 3 lands.
                s = st[b]
                n, jj = j // 4, j % 4
                cw = s["cw"][n]
                for dc in dcs:
                    nc.tensor.matmul(
                        ps_d,
                        cw[:, dc, 128 * jj:128 * (jj + 1)],
                        dw_sb[:, dc, :],
                        start=(dc == 0), stop=(dc == NDC - 1))
                if dcs[-1] == NDC - 1:
                    emit_dense_out(b, j, ps_d)

            # ================= emission schedule =================
            from collections import deque

            # Phase A: batch-0 GEMMs per x-slice quarter; batch-1 x DMAs
            # queued right behind batch-0's.
            emit_x_load(0)
            emit_x_load(1)
            for q in range(SCH):
                for j in range(4 * q, 4 * q + 4):
                    emit_kv(0, j)
                for pr in range(NPAIR):
                    emit_qt(0, pr, q)
                if q == 2:
                    emit_proj_fetch(0, 0)
                    emit_proj_fetch(0, 1)
            for pr in range(NPAIR):
                emit_qdup(0, pr)
            emit_proj(0, 0)
            emit_proj_fetch(0, 2)
            emit_qt(1, 0, 0)
            emit_kt(0, 0)
            emit_proj(0, 1)
            emit_proj_fetch(0, 3)
            emit_qt(1, 0, 1)
            emit_kt(0, 1)
            emit_proj(0, 2)
            emit_qt(1, 0, 2)
            emit_kt(0, 2)
            emit_proj(0, 3)
            emit_qt(1, 0, 3)
            emit_kt(0, 3)
            emit_qdup(1, 0)

            # Phase B: batch-0 attention with batch-1 GEMM units as fillers.
            qt_units = []
            for pr in range(1, NPAIR):
                qt_units += [(emit_qt, (1, pr, n)) for n in range(SCH)]
                qt_units.append((emit_qdup, (1, pr)))
            fill = deque(
                [(emit_kv, (1, j)) for j in range(8)] +
                [(emit_proj_fetch, (1, 0))] +
                [(emit_kv, (1, j)) for j in range(8, NJ)] +
                [(emit_proj_fetch, (1, 1))] +
                qt_units +
                [(emit_proj, (1, 0)), (emit_proj_fetch, (1, 2)),
                 (emit_kt, (1, 0)),
                 (emit_proj, (1, 1)), (emit_proj_fetch, (1, 3)),
                 (emit_kt, (1, 1)),
                 (emit_proj, (1, 2)), (emit_kt, (1, 2)),
                 (emit_proj, (1, 3)), (emit_kt, (1, 3))])

            def popf():
                # emit filler units until one with PE work was emitted
                while fill:
                    f, a = fill.popleft()
                    f(*a)
                    if f not in (emit_proj_fetch, emit_qdup):
                        break

            # scores runs one unit ahead of zat so the ACT exp latency is
            # always covered by the next unit's matmuls even with no fillers.
            prev = None
            for pr in range(NPAIR):
                for n in range(SCH):
                    emit_scores(0, pr, n)
                    popf()
                    if prev is not None:
                        emit_zat(0, *prev)
                        popf()
                    prev = (pr, n)
            emit_zat(0, *prev)
            while fill:
                popf()

            # Phase C: batch-1 attention (scores pipelined one ahead of zat);
            # fillers are batch-0 dense then batch-1 dense as chunks complete.
            # The last chunk's dense is accumulated progressively per pair so
            # only the dc=3 matmuls remain after the final zat.
            fill = deque([(emit_dense, (0, j)) for j in range(NJ)])
            prev = None
            ps_tail = None
            for n in range(SCH):
                for pr in range(NPAIR):
                    emit_scores(1, pr, n)
                    popf()
                    if prev is not None:
                        emit_zat(1, *prev)
                        popf()
                        if prev[1] == SCH - 1 and prev[0] == 2:
                            # pairs 0-2 of the last chunk are done: run their
                            # dense contributions now (dc = pair index).
                            ps_tail = [psB.tile([128, D], f32, tag="ps512",
                                                name=f"ps_tail_{jj}")
                                       for jj in range(4)]
                            for jj, ps_d in enumerate(ps_tail):
                                emit_dense_partial(1, 12 + jj, (0, 1, 2), ps_d)
                    prev = (pr, n)
                if n < SCH - 1:
                    for j in range(4 * n, 4 * n + 4):
                        fill.append((emit_dense, (1, j)))
            emit_zat(1, *prev)
            for jj, ps_d in enumerate(ps_tail):
                emit_dense_partial(1, 12 + jj, (3,), ps_d)
            while fill:
                popf()

    nc.finalize()
    return nc


def _prep_inputs(x, mask, wq, wk, wv, EW, FW, conv_w1, conv_w3, conv_w5, conv_b,
                 dense_w, dense_b, cluster_table):
    """Host-side restructuring -> per-core input maps."""
    bf = ml_dtypes.bfloat16
    x = np.ascontiguousarray(np.asarray(x, np.float32))
    mask = np.asarray(mask)
    counts = np.clip(mask.astype(np.int64).sum(1), 1, S)
    pos = np.asarray(cluster_table)[counts - 1]          # [B, P, C]
    if not (pos == pos[0]).all():
        raise NotImplementedError("per-batch cluster tables not supported")
    p0 = pos[0]                                          # [P, C]

    scale = 1.0 / np.sqrt(np.float32(DEPTH))
    s_idx = p0.ravel()
    c_idx = np.repeat(np.arange(P), C)

    def build_table(W, sc):
        A = np.zeros((H, S + 1, P), np.float32)
        np.add.at(A, (np.arange(H)[:, None], s_idx[None, :], c_idx[None, :]),
                  np.asarray(W, np.float32).reshape(H, P * C) * sc)
        return np.ascontiguousarray(A[:, :S, :])

    AE = build_table(EW, scale)
    AF = build_table(FW, 1.0)
    # pack adjacent heads side by side: [NPAIR, S, 128]
    AE = np.ascontiguousarray(
        AE.reshape(NPAIR, 2, S, P).transpose(0, 2, 1, 3).reshape(NPAIR, S, 128))
    AF = np.ascontiguousarray(
        AF.reshape(NPAIR, 2, S, P).transpose(0, 2, 1, 3).reshape(NPAIR, S, 128))
    # partition-major for fast DMA: [NPAIR, 128, NJ, 128]
    AE = np.ascontiguousarray(
        AE.reshape(NPAIR, NJ, 128, 128).transpose(0, 2, 1, 3))
    AF = np.ascontiguousarray(
        AF.reshape(NPAIR, NJ, 128, 128).transpose(0, 2, 1, 3))



    # conv -> 5 tap matrices (per-head [P, P], duplicated on both halves)
    wp = np.arange(P)[:, None]
    jj = np.arange(P)[None, :]
    ii = wp - jj + 31
    valid = (ii >= 0) & (ii < P)
    ii = np.clip(ii, 0, P - 1)
    M = {t: np.zeros((P, P), np.float32) for t in range(-2, 3)}
    for cw, hk in ((conv_w1, 1), (conv_w3, 3), (conv_w5, 5)):
        cw = np.asarray(cw, np.float32)
        pad = (hk - 1) // 2
        for dy in range(hk):
            filt = cw[dy, :, 0, 0]
            M[dy - pad] += np.where(valid, filt[ii], 0.0) / 3.0
    BDMA = np.zeros((128, 3, P), np.float32)
    for k, t in enumerate((-2, 0, 2)):
        BDMA[:64, k, :] = M[t]
        BDMA[64:, k, :] = M[t]
    BDMB = np.zeros((128, 2, P), np.float32)
    for k, t in enumerate((-1, 1)):
        BDMB[:64, k, :] = M[t]
        BDMB[64:, k, :] = M[t]
    bbar = float(np.asarray(conv_b, np.float32).mean())
    if abs(bbar) > 1e-30:
        raise NotImplementedError("nonzero conv bias not folded")

    ones_bd = np.zeros((128, 128), np.float32)
    ones_bd[:64, :64] = 1.0
    ones_bd[64:, 64:] = 1.0

    # shard + transpose x
    xsh = x.reshape(NCORES, BLOC, S, D)
    in_maps = []
    shared = dict(
        wq=np.asarray(wq, np.float32).astype(bf),
        wk=np.asarray(wk, np.float32).astype(bf),
        wv=np.asarray(wv, np.float32).astype(bf),
        dw=np.asarray(dense_w, np.float32).astype(bf),
        ae=AE.astype(bf), af=AF.astype(bf),
        bdma=BDMA.astype(bf), bdmb=BDMB.astype(bf),
        onesbd=ones_bd.astype(bf),
    )
    for c in range(NCORES):
        m = dict(shared)
        m["xT"] = np.ascontiguousarray(xsh[c].transpose(0, 2, 1)).astype(bf)
        in_maps.append(m)
    return in_maps


def _run(in_maps, trace=False, tmpdir=None):
    from concourse.bass_utils import run_bass_kernel_spmd
    if "nc" not in _CACHE:
        _CACHE["nc"] = _build_nc()
    kw = {}
    if trace:
        _install_ntff_hook()
        kw = dict(trace=True, tmpdir=tmpdir)
    return run_bass_kernel_spmd(_CACHE["nc"], in_maps,
                                core_ids=list(range(NCORES)), **kw)


def _install_ntff_hook():
    import types, importlib.util as ilu
    if "antenv.axon_hooks" in sys.modules:
        return
    spec = ilu.spec_from_file_location(
        "trn_boot_mod", "/root/.axon_site/trn_agent_boot/trn_boot.py")
    tb = ilu.module_from_spec(spec)
    spec.loader.exec_module(tb)
    hook = tb._ntff_profile_via_ctypes("/opt/axon/libaxon_pjrt.so")
    mod = types.ModuleType("antenv.axon_hooks")
    mod.get_axon_ntff_profile_hook = lambda: hook
    import antenv  # noqa: F401
    sys.modules["antenv.axon_hooks"] = mod


def kernel(**inputs) -> np.ndarray:
    in_maps = _prep_inputs(**inputs)
    r = _run(in_maps)
    out = np.concatenate([np.asarray(r.results[c]["out"], np.float32)
                          for c in range(NCORES)], axis=0)
    db = np.asarray(inputs["dense_b"], np.float32)
    if np.any(db):  # dense bias applied host-side (zero in practice)
        out = out + db
    return out



# revision 35
# speedup vs baseline: 1.1448x; 1.0072x over previous
"""Clustered Linformer Attention — Trainium2 Bass kernel, 8 NeuronCores.

Strategy: data-parallel over batch (2 batches/core, no collectives).
Math restructuring (verified vs reference to ~7e-7 in f32):
  - mask is all-ones => cluster c holds positions [32c, 32c+32); the per-head
    gather+einsum projections become  k_proj = AE[h]^T @ k_h  with a host-built
    sparse table AE[h] in [S, P] (score scale folded in), same for v with AF.
  - the 3-kernel conv fusion over scores collapses to 5 "tap" matrices M_t in
    [P, P] (t in -2..2):  scores_conv[s] = sum_t  (q[s+t] @ (k_proj^T @ M_t)).
  - v2: the 5 taps are K-STACKED two-per-matmul: qts stores each head's qT
    twice (partitions 0-63 at shift 0, 64-127 at shift +1), and the tap
    operands T0=[bdt(-2);bdt(-1)], T1=[bdt(0);bdt(+1)], T2=bdt(+2) contract
    over 128/128/64 partitions.  The two heads of a pair run in different PE
    column groups (tile_position col 0 / 64), so the 3 matmuls per head
    overlap pairwise -> ~3 matmul-times for what used to take 5.
  - softmax has no max-subtraction (|scores| <~ 1.6, exp is safe in f32);
    Z = sum_c exp is computed by an all-ones block-diag matmul that also
    broadcasts Z to all 128 partitions, so normalization is one DVE op.

Scheduling: x is DMA'd in column slices so QKV starts early; ae/af cluster
tables are partition-major in DRAM (4KB rows) and prefetched per-pair; each
attention unit is split into scores(+exp) and Z/at(+normalize) halves with a
filler matmul unit between them in PE program order (covers ACT exp latency);
dense output copies alternate ACT/DVE and output DMAs alternate queues.
"""
import sys
import numpy as np
import ml_dtypes

sys.path.insert(0, '/opt/trn_rl_repo')

B, S, D = 16, 2048, 512
H, P, C = 8, 64, 32
DEPTH = D // H           # 64
NCORES = 8
BLOC = B // NCORES       # 2 batches per core
NPAIR = H // 2           # 4 head pairs
SCH = 4                  # s-chunks of 512
SCW = S // SCH           # 512
NJ = S // 128            # 16 s-tiles of 128
NDC = D // 128           # 4 contraction chunks
QW = S + 4               # qts width (2 pad front, 2 back)

_CACHE = {}


def _build_nc():
    import concourse.tile as tile
    from concourse import mybir, bacc

    f32 = mybir.dt.float32
    bf16 = mybir.dt.bfloat16

    nc = bacc.Bacc()
    xT = nc.declare_dram_parameter("xT", [BLOC, D, S], bf16, isOutput=False)
    wq = nc.declare_dram_parameter("wq", [D, D], bf16, isOutput=False)
    wk = nc.declare_dram_parameter("wk", [D, D], bf16, isOutput=False)
    wv = nc.declare_dram_parameter("wv", [D, D], bf16, isOutput=False)
    dw = nc.declare_dram_parameter("dw", [D, D], bf16, isOutput=False)
    # partition-major cluster tables: ae[pr, p, j, c] = AE_pair[pr][128j+p, c]
    ae = nc.declare_dram_parameter("ae", [NPAIR, 128, NJ, 128], bf16,
                                   isOutput=False)
    af = nc.declare_dram_parameter("af", [NPAIR, 128, NJ, 128], bf16,
                                   isOutput=False)
    # per-head tap matrices, duplicated on both partition halves, grouped
    # by K-stack destination: bdma[p, k, c'] = M_{(-2,0,+2)[k]}[p % 64, c']
    # (psum rows 0-63), bdmb[p, k, c'] = M_{(-1,+1)[k]} (psum rows 64-127).
    bdma = nc.declare_dram_parameter("bdma", [128, 3, 64], bf16, isOutput=False)
    bdmb = nc.declare_dram_parameter("bdmb", [128, 2, 64], bf16, isOutput=False)
    onesbd = nc.declare_dram_parameter("onesbd", [128, 128], bf16,
                                       isOutput=False)
    out = nc.declare_dram_parameter("out", [BLOC, S, D], bf16, isOutput=True)

    with tile.TileContext(nc) as tc:
        with tc.tile_pool(name="const", bufs=1) as cpool, \
             tc.tile_pool(name="big", bufs=1) as bigp, \
             tc.tile_pool(name="sm", bufs=4) as smp, \
             tc.tile_pool(name="bd", bufs=8) as bdp, \
             tc.tile_pool(name="ob", bufs=2) as obp, \
             tc.tile_pool(name="psB", bufs=6, space="PSUM") as psB, \
             tc.tile_pool(name="psS", bufs=2, space="PSUM") as psS:

            # ---- constants in SBUF ----
            wq_sb = cpool.tile([128, NDC, D], bf16)
            wk_sb = cpool.tile([128, NDC, D], bf16)
            wv_sb = cpool.tile([128, NDC, D], bf16)
            dw_sb = cpool.tile([128, NDC, D], bf16)
            # dc-sliced weight loads: the first kv matmul only needs
            # wk's dc0 slice (128 KB), not the whole 512 KB tensor, so the
            # ramp starts ~3us earlier.  wk/wv slices interleaved (kv chains
            # consume them in that order), then wq, then dw.
            for dc in range(NDC):
                for t_sb, t_dr in ((wk_sb, wk), (wv_sb, wv)):
                    nc.sync.dma_start(out=t_sb[:, dc, :],
                                      in_=t_dr[128 * dc:128 * (dc + 1), :])
            for t_sb, t_dr in ((wq_sb, wq), (dw_sb, dw)):
                for dc in range(NDC):
                    nc.sync.dma_start(out=t_sb[:, dc, :],
                                      in_=t_dr[128 * dc:128 * (dc + 1), :])
            bdma_sb = cpool.tile([128, 3, 64], bf16)
            nc.sync.dma_start(out=bdma_sb, in_=bdma[:])
            bdmb_sb = cpool.tile([128, 2, 64], bf16)
            nc.sync.dma_start(out=bdmb_sb, in_=bdmb[:])
            ones_sb = cpool.tile([128, 128], bf16)
            nc.sync.dma_start(out=ones_sb, in_=onesbd[:])

            # cluster tables: 8 persistent SBUF tiles (per pair x ae/af,
            # shared by both batches), each fetched ONCE at start with a
            # single contiguous DMA on the sync queue.
            aexp = {}
            for pr in range(NPAIR):
                for tb, srct in (("ae", ae), ("af", af)):
                    t = cpool.tile([128, NJ, 128], bf16, name=f"aexp_{tb}{pr}")
                    aexp[(tb, pr)] = t
                    nc.sync.dma_start(out=t, in_=srct[pr])

            st = [dict() for _ in range(BLOC)]

            def emit_x_load(b):
                # column-sliced so kv/qt of early s-chunks start ASAP.
                # batch 0 feeds phase A from the gpsimd queue; batch 1 rides
                # the sync queue (behind weights+tables, done well before
                # phase B needs it) so the two batches transfer in parallel.
                s = st[b]
                s["xt"] = [bigp.tile([128, S], bf16, tag="xt", bufs=2 * NDC,
                                     name=f"xt_{b}_{dc}")
                           for dc in range(NDC)]
                for q in range(SCH):
                    for dc in range(NDC):
                        # batch 0 feeds phase A from gpsimd (first quarter
                        # split with scalar to halve time-to-first-matmul);
                        # batch 1 rides sync behind weights+tables.
                        if b == 0:
                            eng = nc.scalar if (q == 0 and dc >= 2) \
                                else nc.gpsimd
                        else:
                            eng = nc.sync
                        eng.dma_start(
                            out=s["xt"][dc][:, SCW * q:SCW * (q + 1)],
                            in_=xT[b, 128 * dc:128 * (dc + 1),
                                   SCW * q:SCW * (q + 1)])

            def emit_kv(b, j):
                s = st[b]
                if j == 0:
                    s["knat"] = bigp.tile([128, NJ, D], bf16, tag="knat",
                                          name=f"knat_{b}")
                    s["vnat"] = bigp.tile([128, NJ, D], bf16, tag="vnat",
                                          name=f"vnat_{b}")
                for w_sb, key in ((wk_sb, "knat"), (wv_sb, "vnat")):
                    ps_k = psB.tile([128, D], f32, tag="ps512")
                    for dc in range(NDC):
                        nc.tensor.matmul(
                            ps_k,
                            s["xt"][dc][:, 128 * j:128 * (j + 1)],
                            w_sb[:, dc, :],
                            start=(dc == 0), stop=(dc == NDC - 1))
                    if key == "knat":
                        nc.vector.tensor_copy(out=s[key][:, j, :], in_=ps_k)
                    else:
                        nc.scalar.copy(out=s[key][:, j, :], in_=ps_k)

            def emit_qt(b, pr, n):
                # per-pair duplicated qT tile [128, 2 heads, QW]: partitions
                # 0-63 hold q[d, col-2], partitions 64-127 q[d, col-1].
                # b1's pair tiles ring-reuse b0's as those pairs retire.
                s = st[b]
                if n == 0:
                    t = bigp.tile([128, 2, QW], bf16, tag="qtsp", bufs=5,
                                  name=f"qts_{b}_{pr}")
                    s.setdefault("qtsp", {})[pr] = t
                    nc.vector.memset(t[0:64, :, 0:2], 0.0)
                    nc.vector.memset(t[0:64, :, S + 2:], 0.0)
                    nc.vector.memset(t[64:128, :, 0:1], 0.0)
                    nc.vector.memset(t[64:128, :, S + 1:], 0.0)
                ps_q = psB.tile([128, SCW], f32, tag="ps512")
                for dc in range(NDC):
                    nc.tensor.matmul(
                        ps_q,
                        wq_sb[:, dc, 128 * pr:128 * (pr + 1)],
                        s["xt"][dc][:, SCW * n:SCW * (n + 1)],
                        start=(dc == 0), stop=(dc == NDC - 1))
                qts = s["qtsp"][pr]
                b0 = 2 + SCW * n       # block0 col of s = SCW*n
                # one full-width ACT copy: h0 rows into slot 0 block0;
                # h1 rows land in slot 0's upper half (staging).
                nc.scalar.copy(out=qts[:, 0, b0:b0 + SCW], in_=ps_q)

            def emit_qdup(b, pr):
                # after all 4 chunks of this pair are staged, build the
                # duplicated layouts with 3 full-width SBUF->SBUF DMAs
                # (same queue => in order; gpsimd queue, behind the x feed).
                s = st[b]
                qts = s["qtsp"][pr]
                #  a) h1 block1 <- staged h1 rows (same partitions, col -1)
                nc.gpsimd.dma_start(out=qts[64:128, 1, 1:1 + S],
                                    in_=qts[64:128, 0, 2:2 + S])
                #  b) h1 block0 <- staged h1 rows (cross partition 64->0)
                nc.gpsimd.dma_start(out=qts[0:64, 1, 2:2 + S],
                                    in_=qts[64:128, 0, 2:2 + S])
                #  c) h0 block1 <- h0 block0 (cross 0->64, col -1); clobbers
                #     the staging except its last column...
                nc.gpsimd.dma_start(out=qts[64:128, 0, 1:1 + S],
                                    in_=qts[0:64, 0, 2:2 + S])
                #     ... and its last column must read q(S)=0: copy the
                #     always-zero block1 col 0 (same queue => after a/b).
                nc.gpsimd.dma_start(out=qts[64:128, 0, S + 1:S + 2],
                                    in_=qts[64:128, 0, 0:1])

            def emit_proj_fetch(b, pr):
                pass  # tables are persistent; fetched once at start

            def emit_proj(b, pr):
                s = st[b]
                if pr == 0:
                    s["kp"] = bigp.tile([128, NPAIR, 128], bf16, tag="kpbd",
                                        bufs=2, name=f"kp_{b}")
                    s["vp"] = bigp.tile([128, NPAIR, 128], bf16, tag="vpbd",
                                        bufs=2, name=f"vp_{b}")
                    nc.vector.memset(s["vp"], 0.0)
                for a_sb, key, dstk in ((aexp[("ae", pr)], "knat", "kp"),
                                        (aexp[("af", pr)], "vnat", "vp")):
                    ps_p = psS.tile([128, 128], f32, tag="pssmall")
                    for j in range(NJ):
                        nc.tensor.matmul(
                            ps_p,
                            a_sb[:, j, :],
                            st[b][key][:, j, 128 * pr:128 * (pr + 1)],
                            start=(j == 0), stop=(j == NJ - 1))
                    dst = st[b][dstk]
                    if dstk == "kp":
                        # only diag blocks are ever read (per-head lhsT)
                        nc.vector.tensor_copy(out=dst[:, pr, :], in_=ps_p)
                    else:
                        # vp is used as a block-diag [c,d] operand: keep
                        # off-diag zero.
                        nc.vector.tensor_copy(
                            out=dst[0:64, pr, 0:64], in_=ps_p[0:64, 0:64])
                        nc.vector.tensor_copy(
                            out=dst[64:128, pr, 64:128],
                            in_=ps_p[64:128, 64:128])

            def emit_kt(b, pr):
                # per-head K-stacked tap operands:
                # T[:,0]=[bdt(-2);bdt(-1)], T[:,1]=[bdt(0);bdt(+1)],
                # T[0:64,2]=bdt(+2), where bdt(t) = kp_h^T @ M_t  [d, c'].
                s = st[b]
                if pr == 0:
                    s["bdts"] = {}
                    s["expt"] = {}
                    s["cw"] = {}
                for h2 in (0, 1):
                    hb = 64 * h2
                    h = 2 * pr + h2
                    kp_h = s["kp"][hb:hb + 64, pr, hb:hb + 64]
                    T = bdp.tile([128, 3, 64], bf16, tag="bdts",
                                 name=f"bdts_{b}_{h}")
                    s["bdts"][h] = T
                    ps_b = psS.tile([128, 3, 64], f32, tag="pssmall")
                    nc.tensor.matmul(ps_b[0:64, :, :], kp_h,
                                     bdma_sb[hb:hb + 64, :, :],
                                     start=True, stop=True)
                    nc.tensor.matmul(ps_b[64:128, 0:2, :], kp_h,
                                     bdmb_sb[hb:hb + 64, :, :],
                                     start=True, stop=True)
                    nc.scalar.copy(out=T[:, 0:2, :], in_=ps_b[:, 0:2, :])
                    nc.scalar.copy(out=T[0:64, 2, :], in_=ps_b[0:64, 2, :])

            def emit_scores(b, pr, n):
                # 3 K-stacked tap matmuls per head; heads in different PE
                # column groups so consecutive pairs overlap.
                s = st[b]
                qts = s["qtsp"][pr]
                ps_sc = psB.tile([128, SCW], f32, tag="ps512")
                base = SCW * n
                for h2 in (0, 1):
                    hb = 64 * h2
                    h = 2 * pr + h2
                    T = s["bdts"][h]
                    nc.tensor.matmul(ps_sc[hb:hb + 64, :], T[:, 0, :],
                                     qts[:, h2, base:base + SCW],
                                     start=True, stop=False)
                for h2 in (0, 1):
                    hb = 64 * h2
                    h = 2 * pr + h2
                    T = s["bdts"][h]
                    nc.tensor.matmul(ps_sc[hb:hb + 64, :], T[:, 1, :],
                                     qts[:, h2, base + 2:base + 2 + SCW],
                                     start=False, stop=False)
                for h2 in (0, 1):
                    hb = 64 * h2
                    h = 2 * pr + h2
                    T = s["bdts"][h]
                    nc.tensor.matmul(ps_sc[hb:hb + 64, :], T[0:64, 2, :],
                                     qts[0:64, h2, base + 4:base + 4 + SCW],
                                     start=False, stop=True)
                expt = smp.tile([128, SCW], bf16, tag="expt", bufs=4)
                nc.scalar.activation(
                    out=expt, in_=ps_sc,
                    func=mybir.ActivationFunctionType.Exp)
                s["expt"][(pr, n)] = expt

            def emit_zat(b, pr, n):
                s = st[b]
                expt = s["expt"].pop((pr, n))
                if pr == 0:
                    s["cw"][n] = bigp.tile([128, NPAIR, SCW], bf16,
                                           tag="cwin", bufs=6,
                                           name=f"cw_{b}_{n}")
                ps_z = psB.tile([128, SCW], f32, tag="ps512")
                nc.tensor.matmul(ps_z, ones_sb, expt, start=True, stop=True)
                ps_at = psB.tile([128, SCW], f32, tag="ps512")
                nc.tensor.matmul(ps_at, s["vp"][:, pr, :], expt,
                                 start=True, stop=True)
                rzb = smp.tile([128, SCW], f32, tag="rzb", bufs=1)
                nc.vector.reciprocal_approx_fast(out=rzb, in_=ps_z)
                nc.vector.tensor_mul(
                    out=s["cw"][n][:, pr, :], in0=ps_at, in1=rzb)

            def emit_dense(b, j):
                s = st[b]
                n, jj = j // 4, j % 4
                cw = s["cw"][n]
                ps_d = psB.tile([128, D], f32, tag="ps512")
                for dc in range(NDC):
                    nc.tensor.matmul(
                        ps_d,
                        cw[:, dc, 128 * jj:128 * (jj + 1)],
                        dw_sb[:, dc, :],
                        start=(dc == 0), stop=(dc == NDC - 1))
                emit_dense_out(b, j, ps_d)

            def emit_dense_out(b, j, ps_d):
                obuf = obp.tile([128, D], bf16, tag="obuf", bufs=4)
                if j % 2:
                    nc.vector.tensor_copy(out=obuf, in_=ps_d)
                else:
                    nc.scalar.copy(out=obuf, in_=ps_d)
                eng = (nc.sync, nc.scalar)[j % 2]
                eng.dma_start(out=out[b, 128 * j:128 * (j + 1), :], in_=obuf)

            def emit_dense_partial(b, j, dcs, ps_d):
                # progressive tail dense: accumulate listed dc chunks of
                # output tile j; finish (copy+DMA) when dc 3 lands.
                s = st[b]
                n, jj = j // 4, j % 4
                cw = s["cw"][n]
                for dc in dcs:
                    nc.tensor.matmul(
                        ps_d,
                        cw[:, dc, 128 * jj:128 * (jj + 1)],
                        dw_sb[:, dc, :],
                        start=(dc == 0), stop=(dc == NDC - 1))
                if dcs[-1] == NDC - 1:
                    emit_dense_out(b, j, ps_d)

            # ================= emission schedule =================
            from collections import deque

            # Phase A: batch-0 GEMMs per x-slice quarter; batch-1 x DMAs
            # queued right behind batch-0's.
            emit_x_load(0)
            emit_x_load(1)
            for q in range(SCH):
                for j in range(4 * q, 4 * q + 4):
                    emit_kv(0, j)
                for pr in range(NPAIR):
                    emit_qt(0, pr, q)
                if q == 2:
                    emit_proj_fetch(0, 0)
                    emit_proj_fetch(0, 1)
            for pr in range(NPAIR):
                emit_qdup(0, pr)
            emit_proj(0, 0)
            emit_proj_fetch(0, 2)
            emit_qt(1, 0, 0)
            emit_kt(0, 0)
            emit_proj(0, 1)
            emit_proj_fetch(0, 3)
            emit_qt(1, 0, 1)
            emit_kt(0, 1)
            emit_proj(0, 2)
            emit_qt(1, 0, 2)
            emit_kt(0, 2)
            emit_proj(0, 3)
            emit_qt(1, 0, 3)
            emit_kt(0, 3)
            emit_qdup(1, 0)

            # Phase B: batch-0 attention with batch-1 GEMM units as fillers.
            qt_units = []
            for pr in range(1, NPAIR):
                qt_units += [(emit_qt, (1, pr, n)) for n in range(SCH)]
                qt_units.append((emit_qdup, (1, pr)))
            fill = deque(
                [(emit_kv, (1, j)) for j in range(8)] +
                [(emit_proj_fetch, (1, 0))] +
                [(emit_kv, (1, j)) for j in range(8, NJ)] +
                [(emit_proj_fetch, (1, 1))] +
                qt_units +
                [(emit_proj, (1, 0)), (emit_proj_fetch, (1, 2)),
                 (emit_kt, (1, 0)),
                 (emit_proj, (1, 1)), (emit_proj_fetch, (1, 3)),
                 (emit_kt, (1, 1)),
                 (emit_proj, (1, 2)), (emit_kt, (1, 2)),
                 (emit_proj, (1, 3)), (emit_kt, (1, 3))])

            def popf():
                # emit filler units until one with PE work was emitted
                while fill:
                    f, a = fill.popleft()
                    f(*a)
                    if f not in (emit_proj_fetch, emit_qdup):
                        break

            # scores runs one unit ahead of zat so the ACT exp latency is
            # always covered by the next unit's matmuls even with no fillers.
            prev = None
            for pr in range(NPAIR):
                for n in range(SCH):
                    emit_scores(0, pr, n)
                    popf()
                    if prev is not None:
                        emit_zat(0, *prev)
                        popf()
                    prev = (pr, n)
            emit_zat(0, *prev)
            while fill:
                popf()

            # Phase C: batch-1 attention (scores pipelined one ahead of zat);
            # fillers are batch-0 dense then batch-1 dense as chunks complete.
            # The last chunk's dense is accumulated progressively per pair so
            # only the dc=3 matmuls remain after the final zat.
            fill = deque([(emit_dense, (0, j)) for j in range(NJ)])
            prev = None
            ps_tail = None
            for n in range(SCH):
                for pr in range(NPAIR):
                    emit_scores(1, pr, n)
                    popf()
                    if prev is not None:
                        emit_zat(1, *prev)
                        popf()
                        if prev[1] == SCH - 1 and prev[0] == 2:
                            # pairs 0-2 of the last chunk are done: run their
                            # dense contributions now (dc = pair index).
                            ps_tail = [psB.tile([128, D], f32, tag="ps512",
                                                name=f"ps_tail_{jj}")
                                       for jj in range(4)]
                            for jj, ps_d in enumerate(ps_tail):
                                emit_dense_partial(1, 12 + jj, (0, 1, 2), ps_d)
                    prev = (pr, n)
                if n < SCH - 1:
                    for j in range(4 * n, 4 * n + 4):
                        fill.append((emit_dense, (1, j)))
            emit_zat(1, *prev)
            for jj, ps_d in enumerate(ps_tail):
                emit_dense_partial(1, 12 + jj, (3,), ps_d)
            while fill:
                popf()

    nc.finalize()
    return nc


def _prep_inputs(x, mask, wq, wk, wv, EW, FW, conv_w1, conv_w3, conv_w5, conv_b,
                 dense_w, dense_b, cluster_table):
    """Host-side restructuring -> per-core input maps."""
    bf = ml_dtypes.bfloat16
    x = np.ascontiguousarray(np.asarray(x, np.float32))
    mask = np.asarray(mask)
    counts = np.clip(mask.astype(np.int64).sum(1), 1, S)
    pos = np.asarray(cluster_table)[counts - 1]          # [B, P, C]
    if not (pos == pos[0]).all():
        raise NotImplementedError("per-batch cluster tables not supported")
    p0 = pos[0]                                          # [P, C]

    scale = 1.0 / np.sqrt(np.float32(DEPTH))
    s_idx = p0.ravel()
    c_idx = np.repeat(np.arange(P), C)

    def build_table(W, sc):
        A = np.zeros((H, S + 1, P), np.float32)
        np.add.at(A, (np.arange(H)[:, None], s_idx[None, :], c_idx[None, :]),
                  np.asarray(W, np.float32).reshape(H, P * C) * sc)
        return np.ascontiguousarray(A[:, :S, :])

    AE = build_table(EW, scale)
    AF = build_table(FW, 1.0)
    # pack adjacent heads side by side: [NPAIR, S, 128]
    AE = np.ascontiguousarray(
        AE.reshape(NPAIR, 2, S, P).transpose(0, 2, 1, 3).reshape(NPAIR, S, 128))
    AF = np.ascontiguousarray(
        AF.reshape(NPAIR, 2, S, P).transpose(0, 2, 1, 3).reshape(NPAIR, S, 128))
    # partition-major for fast DMA: [NPAIR, 128, NJ, 128]
    AE = np.ascontiguousarray(
        AE.reshape(NPAIR, NJ, 128, 128).transpose(0, 2, 1, 3))
    AF = np.ascontiguousarray(
        AF.reshape(NPAIR, NJ, 128, 128).transpose(0, 2, 1, 3))



    # conv -> 5 tap matrices (per-head [P, P], duplicated on both halves)
    wp = np.arange(P)[:, None]
    jj = np.arange(P)[None, :]
    ii = wp - jj + 31
    valid = (ii >= 0) & (ii < P)
    ii = np.clip(ii, 0, P - 1)
    M = {t: np.zeros((P, P), np.float32) for t in range(-2, 3)}
    for cw, hk in ((conv_w1, 1), (conv_w3, 3), (conv_w5, 5)):
        cw = np.asarray(cw, np.float32)
        pad = (hk - 1) // 2
        for dy in range(hk):
            filt = cw[dy, :, 0, 0]
            M[dy - pad] += np.where(valid, filt[ii], 0.0) / 3.0
    BDMA = np.zeros((128, 3, P), np.float32)
    for k, t in enumerate((-2, 0, 2)):
        BDMA[:64, k, :] = M[t]
        BDMA[64:, k, :] = M[t]
    BDMB = np.zeros((128, 2, P), np.float32)
    for k, t in enumerate((-1, 1)):
        BDMB[:64, k, :] = M[t]
        BDMB[64:, k, :] = M[t]
    bbar = float(np.asarray(conv_b, np.float32).mean())
    if abs(bbar) > 1e-30:
        raise NotImplementedError("nonzero conv bias not folded")

    ones_bd = np.zeros((128, 128), np.float32)
    ones_bd[:64, :64] = 1.0
    ones_bd[64:, 64:] = 1.0

    # shard + transpose x
    xsh = x.reshape(NCORES, BLOC, S, D)
    in_maps = []
    shared = dict(
        wq=np.asarray(wq, np.float32).astype(bf),
        wk=np.asarray(wk, np.float32).astype(bf),
        wv=np.asarray(wv, np.float32).astype(bf),
        dw=np.asarray(dense_w, np.float32).astype(bf),
        ae=AE.astype(bf), af=AF.astype(bf),
        bdma=BDMA.astype(bf), bdmb=BDMB.astype(bf),
        onesbd=ones_bd.astype(bf),
    )
    for c in range(NCORES):
        m = dict(shared)
        m["xT"] = np.ascontiguousarray(xsh[c].transpose(0, 2, 1)).astype(bf)
        in_maps.append(m)
    return in_maps


def _run(in_maps, trace=False, tmpdir=None):
    from concourse.bass_utils import run_bass_kernel_spmd
    if "nc" not in _CACHE:
        _CACHE["nc"] = _build_nc()
    kw = {}
    if trace:
        _install_ntff_hook()
        kw = dict(trace=True, tmpdir=tmpdir)
    return run_bass_kernel_spmd(_CACHE["nc"], in_maps,
                                core_ids=list(range(NCORES)), **kw)


def _install_ntff_hook():
    import types, importlib.util as ilu
    if "antenv.axon_hooks" in sys.modules:
        return
    spec = ilu.spec_from_file_location(
        "trn_boot_mod", "/root/.axon_site/trn_agent_boot/trn_boot.py")
    tb = ilu.module_from_spec(spec)
    spec.loader.exec_module(tb)
    hook = tb._ntff_profile_via_ctypes("/opt/axon/libaxon_pjrt.so")
    mod = types.ModuleType("antenv.axon_hooks")
    mod.get_axon_ntff_profile_hook = lambda: hook
    import antenv  # noqa: F401
    sys.modules["antenv.axon_hooks"] = mod


def kernel(**inputs) -> np.ndarray:
    in_maps = _prep_inputs(**inputs)
    r = _run(in_maps)
    out = np.concatenate([np.asarray(r.results[c]["out"], np.float32)
                          for c in range(NCORES)], axis=0)
    db = np.asarray(inputs["dense_b"], np.float32)
    if np.any(db):  # dense bias applied host-side (zero in practice)
        out = out + db
    return out

